# revision 1
# baseline (speedup 1.0000x reference)
"""Vocab-sharded AdaptiveSoftmax (log_softmax loss head) on 8 TRN2 NeuronCores.

Reference, for x:[2,512,1024] (T=1024 tokens, H=1024):
  head  = x @ W_head.T          -> cols 0:20000 raw logits + 2 cluster logits
  tail1 = cl0 + log_softmax(x @ W_proj1.T @ W_tail1.T)   (40000 vocab)
  tail2 = cl1 + log_softmax(x @ W_proj2.T @ W_tail2.T)   (140000 vocab)
  out   = concat([head[:, :20000], tail1, tail2], -1)

Sharding: vocab dim of head/tail weights split 8 ways (2500/5000/17500 rows
per core, pre-transposed + bf16-cast on host); x and projections replicated.
log_softmax normalizers = AllReduce(add) of per-token exp-sums (the data
distribution keeps |logits| < ~2, so no max-subtraction is needed).

Per-core phases (bf16 matmuls, f32 PSUM; output written bf16, host upcasts):
  P : proj1T/proj2T (token-transposed, reused as matmul lhsT) + cluster logits
  H : head raw logits -> out (starts the HBM write pipe early)
  T1: logits -> bf16 SBUF stage; two 4-tile-batched AllReduces; bias applied
      in place on the stage (ACT only, so AR-gated ops never sit ahead of
      the PSUM-draining DVE queue); one DMA per 128-token tile
  T2: same per 128-token tile with per-tile AllReduces, software-pipelined
      two tiles deep over 3 stage buffers; tiles 0-1 overlap T1 tail work
Engine split: PE matmuls; DVE psum->stage casts; ACT exp+accum sums, Ln,
bias adds; collectives on TOPSP; big DMAs on the Sync HWDGE queue.
"""

import sys

import numpy as np

if "/opt/trn_rl_repo" not in sys.path:
    sys.path.insert(0, "/opt/trn_rl_repo")

P = 128
T = 1024          # tokens (2*512)
NT = T // P       # 8 token tiles
H = 1024
KO_H = H // P     # 8
VH = 2500         # head vocab shard
V1 = 5000         # tail1 vocab shard
V2 = 17500        # tail2 vocab shard
E1, E2 = 512, 256
KO_1, KO_2 = E1 // P, E2 // P
C = 512           # matmul free-dim sub-block == one f32 PSUM bank
N_CORES = 8
VOUT = VH + V1 + V2   # 25000 per-core out cols

_CACHE = {}


def _build():
    import concourse.bacc as bacc
    import concourse.mybir as mybir
    import concourse.tile as tile
    from contextlib import ExitStack

    bf16 = mybir.dt.bfloat16
    f32 = mybir.dt.float32
    Exp = mybir.ActivationFunctionType.Exp
    Ident = mybir.ActivationFunctionType.Identity
    Ln = mybir.ActivationFunctionType.Ln
    AX = mybir.AxisListType.X

    nc = bacc.Bacc("TRN2", target_bir_lowering=False, debug=False,
                   num_devices=N_CORES)

    xT_d = nc.declare_dram_parameter("xT", [P, KO_H, T], bf16, False)
    whead_d = nc.declare_dram_parameter("wheadT", [P, KO_H, VH], bf16, False)
    wcl_d = nc.declare_dram_parameter("wclT", [P, KO_H, 2], bf16, False)
    wp1_d = nc.declare_dram_parameter("wp1T", [P, KO_H, E1], bf16, False)
    wp2_d = nc.declare_dram_parameter("wp2T", [P, KO_H, E2], bf16, False)
    wt1_d = nc.declare_dram_parameter("wt1T", [P, KO_1, V1], bf16, False)
    wt2_d = nc.declare_dram_parameter("wt2T", [P, KO_2, V2], bf16, False)
    out_d = nc.declare_dram_parameter("out", [T, VOUT], bf16, True)

    out_r = out_d.ap().rearrange("(t p) v -> p t v", p=P)
    rg = [list(range(N_CORES))]

    def segments(total, big=1536):
        res, off = [], 0
        while off < total:
            w = min(big, total - off)
            res.append((off, w))
            off += w
        return res

    with tile.TileContext(nc) as tc:
        with ExitStack() as root:
            pers = root.enter_context(tc.tile_pool(name="pers", bufs=1))
            psum3 = root.enter_context(
                tc.tile_pool(name="psum3", bufs=2, space="PSUM"))
            psum1 = root.enter_context(
                tc.tile_pool(name="psum1", bufs=2, space="PSUM"))
            dram = root.enter_context(
                tc.tile_pool(name="dram", bufs=1, space="DRAM"))
            scratch = root.enter_context(tc.tile_pool(name="scratch", bufs=2))

            # persistent small tiles
            p2T = pers.tile([P, KO_2, T], bf16, name="p2T")
            cl = pers.tile([P, NT, 2], f32, name="cl")
            s1acc = pers.tile([P, NT, 2], f32, name="s1acc")
            s2acc = pers.tile([P, NT, 4], f32, name="s2acc")
            b1 = pers.tile([P, NT], f32, name="b1")
            b2 = pers.tile([P, NT], f32, name="b2")
            s1 = pers.tile([P, NT], f32, name="s1")
            g1 = pers.tile([P, NT], f32, name="g1")
            # shared exp main-output scratch (bf16), single buffer
            exb = scratch.tile([P, 4375], bf16, tag="exb", bufs=1)

            cc1_in = [dram.tile([P, 4], f32, name=f"cc1_in{i}")
                      for i in range(2)]
            cc1_out = [dram.tile([P, 4], f32, name=f"cc1_out{i}",
                                 addr_space="Shared") for i in range(2)]
            cc2_in = [dram.tile([P, 1], f32, name=f"cc2_in{t}")
                      for t in range(NT)]
            cc2_out = [dram.tile([P, 1], f32, name=f"cc2_out{t}",
                                 addr_space="Shared") for t in range(NT)]

            def mm_seg(ps, w, lhsT_sb, ko, t, rhs_sb, voff):
                """Accumulate [128 tokens, w] logits into psum ps for token
                tile t: contraction over ko*128, rhs columns voff:voff+w.
                k-outer so the stationary operand is reused across the
                consecutive sub-block matmuls."""
                for k in range(ko):
                    for sub in range(0, w, C):
                        sw = min(C, w - sub)
                        nc.tensor.matmul(
                            ps[:, sub:sub + sw],
                            lhsT_sb[:, k, t * P:(t + 1) * P],
                            rhs_sb[:, k, voff + sub:voff + sub + sw],
                            start=(k == 0), stop=(k == ko - 1))

            def mk_psum(w):
                if w > 512:
                    return psum3.tile([P, 1536], f32, tag="mm3", name="ps3")
                return psum1.tile([P, 512], f32, tag="mm1", name="ps1")

            # ================= Phase P =================
            wt1_pool = tc.alloc_tile_pool(name="wt1p", bufs=1)
            wt1 = wt1_pool.tile([P, KO_1, V1], bf16, name="wt1")
            xT_pool = tc.alloc_tile_pool(name="xTp", bufs=1)
            xT = xT_pool.tile([P, KO_H, T], bf16, name="xT")
            p1T_pool = tc.alloc_tile_pool(name="p1Tp", bufs=1, side="right")
            p1T = p1T_pool.tile([P, KO_1, T], bf16, name="p1T")
            whead_pool = tc.alloc_tile_pool(name="wheadp", bufs=1,
                                            side="right")
            whead = whead_pool.tile([P, KO_H, VH], bf16, name="whead")
            wp_pool = tc.alloc_tile_pool(name="wpp", bufs=1, side="right")
            wp1 = wp_pool.tile([P, KO_H, E1], bf16, name="wp1")
            wp2 = wp_pool.tile([P, KO_H, E2], bf16, name="wp2")
            wcl = wp_pool.tile([P, KO_H, 2], bf16, name="wcl")

            nc.sync.dma_start(xT[:], xT_d[:])
            nc.sync.dma_start(wp1[:], wp1_d[:])
            nc.sync.dma_start(wp2[:], wp2_d[:])
            nc.sync.dma_start(wcl[:], wcl_d[:])
            nc.sync.dma_start(whead[:], whead_d[:])   # needed for H
            nc.sync.dma_start(wt1[:], wt1_d[:])       # needed for T1

            for proj_sb, wp_sb, ko in ((p1T, wp1, KO_1), (p2T, wp2, KO_2)):
                for e in range(ko):
                    for th in range(2):
                        ps = psum1.tile([P, 512], f32, tag="mm1")
                        for k in range(KO_H):
                            nc.tensor.matmul(
                                ps[:],
                                wp_sb[:, k, e * P:(e + 1) * P],
                                xT[:, k, th * 512:(th + 1) * 512],
                                start=(k == 0), stop=(k == KO_H - 1))
                        nc.vector.tensor_copy(
                            proj_sb[:, e, th * 512:(th + 1) * 512], ps[:])
            for t in range(NT):
                ps = psum1.tile([P, 512], f32, tag="mm1")
                for k in range(KO_H):
                    nc.tensor.matmul(
                        ps[:, :2], xT[:, k, t * P:(t + 1) * P], wcl[:, k, :],
                        start=(k == 0), stop=(k == KO_H - 1))
                nc.vector.tensor_copy(cl[:, t, :], ps[:, :2])
            wp_pool.release()

            # ================= Phase H: head raw logits =================
            # First phase out so the HBM write pipe starts early.
            headout_pool = tc.alloc_tile_pool(name="headoutp", bufs=2)
            HSEGS = segments(VH)
            for t in range(NT):
                ho = headout_pool.tile([P, VH], bf16, tag="ho")
                for si, (off, w) in enumerate(HSEGS):
                    ps = mk_psum(w)
                    mm_seg(ps, w, xT, KO_H, t, whead, off)
                    if si % 2 == 0:
                        nc.vector.tensor_copy(ho[:, off:off + w], ps[:, :w])
                    else:
                        nc.scalar.copy(ho[:, off:off + w], ps[:, :w])
                nc.sync.dma_start(out_r[:, t, 0:VH], ho[:])
            headout_pool.release()
            xT_pool.release()
            whead_pool.release()

            # ========== Phase T1: tail1, two 4-tile AllReduce batches =======
            T1SEGS = segments(V1)
            stage1_pool = tc.alloc_tile_pool(name="stage1", bufs=5,
                                             side="right")
            stg1 = {}

            def t1_compute(t):
                stg = stage1_pool.tile([P, V1], bf16, tag="stg1",
                                       name=f"stg1_{t}")
                stg1[t] = stg
                for si, (off, w) in enumerate(T1SEGS):
                    ps = mk_psum(w)
                    mm_seg(ps, w, p1T, KO_1, t, wt1, off)
                    nc.vector.tensor_copy(stg[:, off:off + w], ps[:, :w])
                for h in range(2):
                    nc.scalar.activation(
                        exb[:, :2500], stg[:, h * 2500:(h + 1) * 2500],
                        Exp, accum_out=s1acc[:, t, h:h + 1])
                nc.vector.reduce_sum(s1[:, t:t + 1], s1acc[:, t, :], axis=AX)

            def t1_ar(i):  # i = batch 0 (tiles 0-3) or 1 (tiles 4-7)
                nc.gpsimd.dma_start(cc1_in[i][:], s1[:, 4 * i:4 * i + 4])
                nc.gpsimd.collective_compute(
                    "AllReduce", mybir.AluOpType.add, replica_groups=rg,
                    ins=[cc1_in[i][:].opt()], outs=[cc1_out[i][:].opt()])

            def t1_bias(i):
                nc.sync.dma_start(g1[:, 4 * i:4 * i + 4], cc1_out[i][:])
                lng = scratch.tile([P, 4], f32, tag="lng1")
                nc.scalar.activation(lng[:], g1[:, 4 * i:4 * i + 4], Ln)
                nc.vector.tensor_sub(out=b1[:, 4 * i:4 * i + 4],
                                     in0=cl[:, 4 * i:4 * i + 4, 0],
                                     in1=lng[:])

            def t1_finalize(t):
                # ACT only: keeps the AR-gated ops out of the DVE queue that
                # drains PSUM for the next phase's matmuls.
                for h in range(2):
                    src = stg1[t][:, h * 2500:(h + 1) * 2500]
                    nc.scalar.activation(src, src, Ident,
                                         bias=b1[:, t:t + 1])
                nc.sync.dma_start(out_r[:, t, VH:VH + V1], stg1[t][:])

            for t in range(4):
                t1_compute(t)
            t1_ar(0)
            t1_compute(4)
            t1_bias(0)
            t1_finalize(0)
            t1_compute(5)
            t1_finalize(1)
            t1_compute(6)
            t1_finalize(2)
            t1_compute(7)
            t1_ar(1)
            wt1_pool.release()

            # ============ Phase T2: tail2, staged single-pass ============
            # wt2 + two stage buffers fit as soon as wt1 frees, so T2 tiles
            # 0-1 run while T1's AllReduce-B and output tail complete; the
            # third stage buffer reuses T1's stage space once it drains.
            T2SEGS = segments(V2)
            with ExitStack() as t2s:
                wt2_pool = t2s.enter_context(tc.tile_pool(name="wt2p",
                                                          bufs=1))
                wt2 = wt2_pool.tile([P, KO_2, V2], bf16, name="wt2")
                for off, w in T2SEGS:
                    nc.sync.dma_start(wt2[:, :, off:off + w],
                                      wt2_d[:, :, off:off + w])
                sp = [t2s.enter_context(tc.tile_pool(name="s2p0", bufs=1)),
                      t2s.enter_context(tc.tile_pool(name="s2p1", bufs=1)),
                      None]
                stg2 = {}

                def t2_compute(t):
                    stg = sp[t % 3].tile([P, V2], bf16, name=f"stg2_{t}",
                                         tag="s")
                    stg2[t] = stg
                    for si, (off, w) in enumerate(T2SEGS):
                        ps = mk_psum(w)
                        mm_seg(ps, w, p2T, KO_2, t, wt2, off)
                        nc.vector.tensor_copy(stg[:, off:off + w], ps[:, :w])
                    for h in range(4):
                        nc.scalar.activation(
                            exb[:], stg[:, h * 4375:(h + 1) * 4375], Exp,
                            accum_out=s2acc[:, t, h:h + 1])
                    s2t = scratch.tile([P, 1], f32, tag="s1t")
                    nc.vector.reduce_sum(s2t[:], s2acc[:, t, :], axis=AX)
                    nc.gpsimd.dma_start(cc2_in[t][:], s2t[:])
                    nc.gpsimd.collective_compute(
                        "AllReduce", mybir.AluOpType.add, replica_groups=rg,
                        ins=[cc2_in[t][:].opt()], outs=[cc2_out[t][:].opt()])

                def t2_finalize(t):
                    g2t = scratch.tile([P, 1], f32, tag="g1t")
                    nc.sync.dma_start(g2t[:], cc2_out[t][:])
                    lng = scratch.tile([P, 1], f32, tag="lng")
                    nc.scalar.activation(lng[:], g2t[:], Ln)
                    nc.scalar.activation(b2[:, t:t + 1], lng[:], Ident,
                                         bias=cl[:, t, 1:2], scale=-1.0)
                    for h in range(4):
                        sl = stg2[t][:, h * 4375:(h + 1) * 4375]
                        if h < 2:
                            nc.vector.tensor_scalar_add(sl, sl, b2[:, t:t + 1])
                        else:
                            nc.scalar.activation(sl, sl, Ident,
                                                 bias=b2[:, t:t + 1])
                    nc.sync.dma_start(out_r[:, t, VH + V1:VOUT], stg2[t][:])

                t2_compute(0)
                t1_bias(1)
                t2_compute(1)
                for t in range(4, NT):
                    t1_finalize(t)
                t1_finalize(3)
                stage1_pool.release()
                p1T_pool.release()
                sp[2] = t2s.enter_context(
                    tc.tile_pool(name="s2p2", bufs=1, side="right"))
                for t in range(2, NT):
                    t2_finalize(t - 2)
                    t2_compute(t)
                t2_finalize(NT - 2)
                t2_finalize(NT - 1)

    nc.compile()
    return nc


def _get_nc():
    if "nc" not in _CACHE:
        _CACHE["nc"] = _build()
    return _CACHE["nc"]


def _prep_inputs(x, W_head, W_proj1, W_tail1, W_proj2, W_tail2):
    import concourse.mybir as mybir
    bf16 = mybir.dt.np(mybir.dt.bfloat16)

    def kxn(w):  # [N, K] weight -> [128, K//128, N] (K on partitions)
        n, k = w.shape
        return np.ascontiguousarray(
            w.T.reshape(k // P, P, n).transpose(1, 0, 2)).astype(bf16)

    x2 = x.reshape(T, H)
    xT = np.ascontiguousarray(
        x2.T.reshape(KO_H, P, T).transpose(1, 0, 2)).astype(bf16)
    wcl = kxn(W_head[20000:20002])
    wp1 = kxn(W_proj1)
    wp2 = kxn(W_proj2)

    in_maps = []
    for i in range(N_CORES):
        in_maps.append({
            "xT": xT,
            "wheadT": kxn(W_head[i * VH:(i + 1) * VH]),
            "wclT": wcl,
            "wp1T": wp1,
            "wp2T": wp2,
            "wt1T": kxn(W_tail1[i * V1:(i + 1) * V1]),
            "wt2T": kxn(W_tail2[i * V2:(i + 1) * V2]),
        })
    return in_maps


def _assemble(outs):
    final = np.empty((T, 200000), dtype=np.float32)
    for i in range(N_CORES):
        o = np.asarray(outs[i]["out"])
        final[:, i * VH:(i + 1) * VH] = o[:, :VH]
        final[:, 20000 + i * V1:20000 + (i + 1) * V1] = o[:, VH:VH + V1]
        final[:, 60000 + i * V2:60000 + (i + 1) * V2] = o[:, VH + V1:]
    return final.reshape(2, 512, 200000)


def _run(inputs, trace=False, tmpdir=None):
    from concourse import bass_utils
    nc = _get_nc()
    in_maps = _prep_inputs(**inputs)
    res = bass_utils.run_bass_kernel_spmd(
        nc, in_maps, core_ids=list(range(N_CORES)), trace=trace,
        tmpdir=tmpdir)
    return _assemble(res.results), res


def kernel(**inputs):
    inputs = {k: np.asarray(v) for k, v in inputs.items()}
    out, _ = _run(inputs, trace=False)
    return out



# revision 10
# speedup vs baseline: 1.1624x; 1.1624x over previous
"""Vocab-sharded AdaptiveSoftmax (log_softmax loss head) on 8 TRN2 NeuronCores.

Reference, for x:[2,512,1024] (T=1024 tokens, H=1024):
  head  = x @ W_head.T          -> cols 0:20000 raw logits + 2 cluster logits
  tail1 = cl0 + log_softmax(x @ W_proj1.T @ W_tail1.T)   (40000 vocab)
  tail2 = cl1 + log_softmax(x @ W_proj2.T @ W_tail2.T)   (140000 vocab)
  out   = concat([head[:, :20000], tail1, tail2], -1)

Sharding: vocab dim of head/tail weights split 8 ways (2500/5000/17500 rows
per core, pre-transposed, x32-scaled + fp8e4-cast on host); x replicated.
log_softmax normalizers = AllReduce(add) of per-token exp-sums (the data
distribution keeps |logits| < ~3, so no max-subtraction is needed).

All matmuls run fp8e4 with DoubleRow perf mode (2 contraction rows per
pass): weights are scaled x32 into fp8's normal range, and the 1/32
de-scale rides for free on the activation-engine `scale` operand.

Per-core phases:
  P : proj2/proj1 (token-transposed fp8, reused as matmul lhsT) + cl logits
  T2: per 128-token tile: 12 x 1536-col PSUM segs; the first 9 drain via
      DVE (x1/32 -> bf16 raw-logit stage) while ACT computes exp+accum
      sums from PSUM; the last 3 drain via a single ACT Exp (stage holds
      exp, accum gives the sum -> finalize recovers logits via Ln(scale*x)
      with scale = exp(bias)). Per-tile AllReduce, finalize lag 2 tiles,
      3 stage buffers. This splits the elementwise work so ACT and DVE
      both run ~full tilt.
  T1: same split per tile (2 DVE segs / 2 ACT segs), ONE batched
      AllReduce over all 8 tiles; finalizes overlap the head phase.
  H : head raw logits (DVE + ACT drains) -> out; covers T1's AR latency.
Engine split: PE fp8-DR matmuls; DVE psum drains + A-region bias adds
(4x-mode tensor_scalar); ACT exps, Ln finalizes, proj drains; collectives
on TOPSP; big DMAs on the Sync HWDGE queue.
"""

import sys

import numpy as np

if "/opt/trn_rl_repo" not in sys.path:
    sys.path.insert(0, "/opt/trn_rl_repo")

P = 128
T = 1024          # tokens (2*512)
NT = T // P       # 8 token tiles
H = 1024
KO_H = H // P     # 8
VH = 2500         # head vocab shard
VHp = 2512        # padded to %16 for DoubleRow rhs step
V1 = 5000         # tail1 vocab shard
V1p = 5008
V2 = 17500        # tail2 vocab shard
V2p = 17504
E1, E2 = 512, 256
KO_1, KO_2 = E1 // P, E2 // P
C = 512           # matmul free-dim sub-block == one f32 PSUM bank
N_CORES = 8
VOUT = VH + V1 + V2   # 25000 per-core out cols
WSC = 32.0        # host-side weight scale into fp8 normal range
ISC = 1.0 / WSC

# segment tables (psum tiles)
T2SEGS = [(i * 1536, 1536) for i in range(11)] + [(11 * 1536, V2p - 11 * 1536)]
T1SEGS = [(0, 1536), (1536, 1536), (3072, 1536), (4608, V1p - 4608)]
HSEGS = [(0, 1536), (1536, VHp - 1536)]
NB2 = 9           # tail2: segs [0,NB2) drain via DVE, rest via ACT Exp
NB1 = 2           # tail1: same split
AW2 = T2SEGS[NB2][0]      # 13824: start of the exp-staged region
AW1 = T1SEGS[NB1][0]      # 3072

_CACHE = {}


def _build():
    import concourse.bacc as bacc
    import concourse.mybir as mybir
    import concourse.tile as tile
    from contextlib import ExitStack

    f8 = mybir.dt.float8e4
    bf16 = mybir.dt.bfloat16
    f32 = mybir.dt.float32
    Exp = mybir.ActivationFunctionType.Exp
    Ident = mybir.ActivationFunctionType.Identity
    Ln = mybir.ActivationFunctionType.Ln
    DR = mybir.MatmulPerfMode.DoubleRow
    AX = mybir.AxisListType.X

    nc = bacc.Bacc("TRN2", target_bir_lowering=False, debug=False,
                   num_devices=N_CORES)

    xT_d = nc.declare_dram_parameter("xT", [P, KO_H, T], f8, False)
    whead_d = nc.declare_dram_parameter("wheadT", [P, KO_H, VHp], f8, False)
    wcl_d = nc.declare_dram_parameter("wclT", [P, KO_H, 2], f8, False)
    wp1_d = nc.declare_dram_parameter("wp1T", [P, KO_H, E1], f8, False)
    wp2_d = nc.declare_dram_parameter("wp2T", [P, KO_H, E2], f8, False)
    wt1_d = nc.declare_dram_parameter("wt1T", [P, KO_1, V1p], f8, False)
    wt2_d = nc.declare_dram_parameter("wt2T", [P, KO_2, V2p], f8, False)
    out_d = nc.declare_dram_parameter("out", [T, VOUT], bf16, True)

    out_r = out_d.ap().rearrange("(t p) v -> p t v", p=P)
    rg = [list(range(N_CORES))]

    with tile.TileContext(nc) as tc:
        with ExitStack() as root:
            pers = root.enter_context(tc.tile_pool(name="pers", bufs=1))
            psum3 = root.enter_context(
                tc.tile_pool(name="psum3", bufs=2, space="PSUM"))
            psum1 = root.enter_context(
                tc.tile_pool(name="psum1", bufs=2, space="PSUM"))
            dram = root.enter_context(
                tc.tile_pool(name="dram", bufs=1, space="DRAM"))
            scratch = root.enter_context(tc.tile_pool(name="scratch", bufs=2))

            # persistent small tiles
            p1T = pers.tile([P, KO_1, T], f8, name="p1T")
            p2T = pers.tile([P, KO_2, T], f8, name="p2T")
            cl = pers.tile([P, NT, 2], f32, name="cl")
            s1acc = pers.tile([P, NT, 4], f32, name="s1acc")
            s2acc = pers.tile([P, NT, 12], f32, name="s2acc")
            s1 = pers.tile([P, NT], f32, name="s1")
            s2 = pers.tile([P, NT], f32, name="s2")
            g1 = pers.tile([P, NT], f32, name="g1")
            g2 = pers.tile([P, NT], f32, name="g2")
            b1 = pers.tile([P, NT], f32, name="b1")
            b2 = pers.tile([P, NT], f32, name="b2")
            sc1 = pers.tile([P, NT], f32, name="sc1")
            sc2 = pers.tile([P, NT], f32, name="sc2")
            # shared exp discard scratch (bf16), single buffer
            exb = scratch.tile([P, 1536], bf16, tag="exb", bufs=1)

            cc1_in = dram.tile([P, NT], f32, name="cc1_in")
            cc1_out = dram.tile([P, NT], f32, name="cc1_out",
                                addr_space="Shared")
            cc2_in = [dram.tile([P, 1], f32, name=f"cc2_in{t}")
                      for t in range(NT)]
            cc2_out = [dram.tile([P, 1], f32, name=f"cc2_out{t}",
                                 addr_space="Shared") for t in range(NT)]

            def mm_seg(ps, w, lhsT_sb, kop, t, rhs_sb, voff):
                """Accumulate [128 tokens, w] logits (x32 scale) into psum ps
                for token tile t via DoubleRow fp8: kop k-pairs, rhs columns
                voff:voff+w."""
                for kk in range(kop):
                    for sub in range(0, w, C):
                        sw = min(C, w - sub)
                        nc.tensor.matmul(
                            ps[:, sub:sub + sw],
                            lhsT_sb[:, 2 * kk:2 * kk + 2, t * P:(t + 1) * P],
                            rhs_sb[:, 2 * kk:2 * kk + 2,
                                   voff + sub:voff + sub + sw],
                            start=(kk == 0), stop=(kk == kop - 1),
                            perf_mode=DR)

            # ================= Phase P =================
            wp_pool = tc.alloc_tile_pool(name="wpp", bufs=1, side="right")
            wp1 = wp_pool.tile([P, KO_H, E1], f8, name="wp1")
            wp2 = wp_pool.tile([P, KO_H, E2], f8, name="wp2")
            wcl = wp_pool.tile([P, KO_H, 2], f8, name="wcl")
            xT_pool = tc.alloc_tile_pool(name="xTp", bufs=1)
            xT = xT_pool.tile([P, KO_H, T], f8, name="xT")
            wt2_pool = tc.alloc_tile_pool(name="wt2p", bufs=1)
            wt2 = wt2_pool.tile([P, KO_2, V2p], f8, name="wt2")

            nc.sync.dma_start(wp2[:], wp2_d[:])
            nc.sync.dma_start(wcl[:], wcl_d[:])
            nc.sync.dma_start(xT[:], xT_d[:])
            nc.sync.dma_start(wp1[:], wp1_d[:])
            for off, w in T2SEGS:
                nc.sync.dma_start(wt2[:, :, off:off + w],
                                  wt2_d[:, :, off:off + w])

            for proj_sb, wp_sb, ko in ((p2T, wp2, KO_2), (p1T, wp1, KO_1)):
                for e in range(ko):
                    for th in range(2):
                        ps = psum1.tile([P, 512], f32, tag="mm1")
                        for kk in range(KO_H // 2):
                            nc.tensor.matmul(
                                ps[:],
                                wp_sb[:, 2 * kk:2 * kk + 2,
                                      e * P:(e + 1) * P],
                                xT[:, 2 * kk:2 * kk + 2,
                                   th * 512:(th + 1) * 512],
                                start=(kk == 0), stop=(kk == KO_H // 2 - 1),
                                perf_mode=DR)
                        nc.scalar.activation(
                            proj_sb[:, e, th * 512:(th + 1) * 512], ps[:],
                            Ident, scale=ISC)
            for t in range(NT):
                ps = psum1.tile([P, 512], f32, tag="mm1")
                for kk in range(KO_H // 2):
                    nc.tensor.matmul(
                        ps[:, :2],
                        xT[:, 2 * kk:2 * kk + 2, t * P:(t + 1) * P],
                        wcl[:, 2 * kk:2 * kk + 2, :],
                        start=(kk == 0), stop=(kk == KO_H // 2 - 1),
                        perf_mode=DR)
                nc.vector.tensor_scalar_mul(cl[:, t, :], ps[:, :2], ISC)
            wp_pool.release()

            # ================= Phase T2 =================
            sp2 = tc.alloc_tile_pool(name="sp2", bufs=4, side="right")
            stg2 = {}

            def t2_compute(t, mid=None):
                stg = sp2.tile([P, V2], bf16, name=f"stg2_{t}", tag="s")
                stg2[t] = stg
                for si, (off, w) in enumerate(T2SEGS):
                    if mid is not None and si == 6:
                        mid()
                    ps = psum3.tile([P, 1536], f32, tag="mm3")
                    mm_seg(ps, w, p2T, KO_2 // 2, t, wt2, off)
                    wt = min(w, V2 - off)
                    if si < NB2:
                        nc.vector.tensor_scalar_mul(
                            stg[:, off:off + w], ps[:, :w], ISC)
                        nc.scalar.activation(
                            exb[:, :wt], ps[:, :wt], Exp, scale=ISC,
                            accum_out=s2acc[:, t, si:si + 1])
                    else:
                        nc.scalar.activation(
                            stg[:, off:off + wt], ps[:, :wt], Exp, scale=ISC,
                            accum_out=s2acc[:, t, si:si + 1])
                nc.vector.reduce_sum(s2[:, t:t + 1], s2acc[:, t, :], axis=AX)
                nc.gpsimd.dma_start(cc2_in[t][:], s2[:, t:t + 1])
                nc.gpsimd.collective_compute(
                    "AllReduce", mybir.AluOpType.add, replica_groups=rg,
                    ins=[cc2_in[t][:].opt()], outs=[cc2_out[t][:].opt()])

            def t2_finalize(t):
                nc.sync.dma_start(g2[:, t:t + 1], cc2_out[t][:])
                lng = scratch.tile([P, 1], f32, tag="lng")
                nc.scalar.activation(lng[:], g2[:, t:t + 1], Ln)
                nc.scalar.activation(b2[:, t:t + 1], lng[:], Ident,
                                     bias=cl[:, t, 1:2], scale=-1.0)
                nc.scalar.activation(sc2[:, t:t + 1], b2[:, t:t + 1], Exp)
                nc.vector.tensor_scalar_add(stg2[t][:, :AW2],
                                            stg2[t][:, :AW2], b2[:, t:t + 1])
                nc.scalar.activation(stg2[t][:, AW2:V2], stg2[t][:, AW2:V2],
                                     Ln, scale=sc2[:, t:t + 1])
                nc.sync.dma_start(out_r[:, t, VH + V1:VOUT], stg2[t][:, :V2])

            t2_compute(0)
            t2_compute(1)
            for t in range(2, NT):
                t2_compute(t, mid=(lambda tt=t: t2_finalize(tt - 2)))
            wt2_pool.release()

            # ================= Phase T1 =================
            # wt1 goes into wt2's freed space; tiles 0-1 stage while sp2 is
            # still alive, the rest after its release.
            wt1_pool = tc.alloc_tile_pool(name="wt1p", bufs=1)
            wt1 = wt1_pool.tile([P, KO_1, V1p], f8, name="wt1")
            nc.sync.dma_start(wt1[:], wt1_d[:])

            sp1a = tc.alloc_tile_pool(name="s1pa", bufs=2)
            stg1 = {}

            def t1_compute(t, pool):
                stg = pool.tile([P, V1], bf16, name=f"stg1_{t}", tag="s1")
                stg1[t] = stg
                for si, (off, w) in enumerate(T1SEGS):
                    if w > 512:
                        ps = psum3.tile([P, 1536], f32, tag="mm3")
                    else:
                        ps = psum1.tile([P, 512], f32, tag="mm1")
                    mm_seg(ps, w, p1T, KO_1 // 2, t, wt1, off)
                    wt = min(w, V1 - off)
                    if si < NB1:
                        nc.vector.tensor_scalar_mul(
                            stg[:, off:off + w], ps[:, :w], ISC)
                        nc.scalar.activation(
                            exb[:, :wt], ps[:, :wt], Exp, scale=ISC,
                            accum_out=s1acc[:, t, si:si + 1])
                    else:
                        nc.scalar.activation(
                            stg[:, off:off + wt], ps[:, :wt], Exp, scale=ISC,
                            accum_out=s1acc[:, t, si:si + 1])
                nc.vector.reduce_sum(s1[:, t:t + 1], s1acc[:, t, :], axis=AX)

            t2_finalize(6)
            t1_compute(0, sp1a)
            t2_finalize(7)
            t1_compute(1, sp1a)
            sp2.release()
            sp1b = tc.alloc_tile_pool(name="s1pb", bufs=6)
            whead_pool = tc.alloc_tile_pool(name="wheadp", bufs=1,
                                            side="right")
            whead = whead_pool.tile([P, KO_H, VHp], f8, name="whead")
            nc.sync.dma_start(whead[:], whead_d[:])
            for t in range(2, NT):
                t1_compute(t, sp1b)
            # one batched AllReduce for all 8 t1 tiles
            nc.gpsimd.dma_start(cc1_in[:], s1[:, :])
            nc.gpsimd.collective_compute(
                "AllReduce", mybir.AluOpType.add, replica_groups=rg,
                ins=[cc1_in[:].opt()], outs=[cc1_out[:].opt()])

            def t1_bias():
                nc.sync.dma_start(g1[:, :], cc1_out[:])
                lng8 = scratch.tile([P, NT], f32, tag="lng8")
                nc.scalar.activation(lng8[:], g1[:, :], Ln)
                nc.vector.tensor_sub(out=b1[:, :], in0=cl[:, :, 0],
                                     in1=lng8[:])
                nc.scalar.activation(sc1[:, :], b1[:, :], Exp)

            def t1_finalize(t):
                nc.vector.tensor_scalar_add(stg1[t][:, :AW1],
                                            stg1[t][:, :AW1], b1[:, t:t + 1])
                nc.scalar.activation(stg1[t][:, AW1:V1], stg1[t][:, AW1:V1],
                                     Ln, scale=sc1[:, t:t + 1])
                nc.sync.dma_start(out_r[:, t, VH:VH + V1], stg1[t][:, :V1])

            # ================= Phase H =================
            headout_pool = tc.alloc_tile_pool(name="headoutp", bufs=3)

            def h_tile(t):
                ho = headout_pool.tile([P, VHp], bf16, tag="ho")
                for si, (off, w) in enumerate(HSEGS):
                    ps = psum3.tile([P, 1536], f32, tag="mm3")
                    mm_seg(ps, w, xT, KO_H // 2, t, whead, off)
                    if si == 0:
                        nc.vector.tensor_scalar_mul(
                            ho[:, off:off + w], ps[:, :w], ISC)
                    else:
                        nc.scalar.activation(ho[:, off:off + w], ps[:, :w],
                                             Ident, scale=ISC)
                nc.sync.dma_start(out_r[:, t, 0:VH], ho[:, :VH])

            h_tile(0)
            h_tile(1)
            h_tile(2)
            t1_bias()
            h_tile(3)
            for t in range(NT):
                t1_finalize(t)
                if t + 4 < NT:
                    h_tile(t + 4)

            # unwind pools (stack order per side)
            headout_pool.release()
            sp1b.release()
            sp1a.release()
            wt1_pool.release()
            xT_pool.release()
            whead_pool.release()

    nc.compile()
    return nc


def _get_nc():
    if "nc" not in _CACHE:
        _CACHE["nc"] = _build()
    return _CACHE["nc"]


def _prep_inputs(x, W_head, W_proj1, W_tail1, W_proj2, W_tail2):
    import concourse.mybir as mybir
    f8np = mybir.dt.np(mybir.dt.float8e4)

    def kxn8(w, pad_to=None, scale=WSC):
        # [N, K] weight -> [128, K//128, Np] fp8 (K on partitions), x scale
        n, k = w.shape
        a = np.ascontiguousarray(
            w.T.reshape(k // P, P, n).transpose(1, 0, 2)) * scale
        if pad_to is not None and pad_to != n:
            a = np.concatenate(
                [a, np.zeros((P, k // P, pad_to - n), np.float32)], axis=2)
        return a.astype(f8np)

    x2 = np.asarray(x, np.float32).reshape(T, H)
    xT = np.ascontiguousarray(
        x2.T.reshape(KO_H, P, T).transpose(1, 0, 2)).astype(f8np)
    wcl = kxn8(W_head[20000:20002])
    wp1 = kxn8(W_proj1)
    wp2 = kxn8(W_proj2)

    in_maps = []
    for i in range(N_CORES):
        in_maps.append({
            "xT": xT,
            "wheadT": kxn8(W_head[i * VH:(i + 1) * VH], VHp),
            "wclT": wcl,
            "wp1T": wp1,
            "wp2T": wp2,
            "wt1T": kxn8(W_tail1[i * V1:(i + 1) * V1], V1p),
            "wt2T": kxn8(W_tail2[i * V2:(i + 1) * V2], V2p),
        })
    return in_maps


def _assemble(outs):
    final = np.empty((T, 200000), dtype=np.float32)
    for i in range(N_CORES):
        o = np.asarray(outs[i]["out"])
        final[:, i * VH:(i + 1) * VH] = o[:, :VH]
        final[:, 20000 + i * V1:20000 + (i + 1) * V1] = o[:, VH:VH + V1]
        final[:, 60000 + i * V2:60000 + (i + 1) * V2] = o[:, VH + V1:]
    return final.reshape(2, 512, 200000)


def _run(inputs, trace=False, tmpdir=None):
    from concourse import bass_utils
    nc = _get_nc()
    in_maps = _prep_inputs(**inputs)
    res = bass_utils.run_bass_kernel_spmd(
        nc, in_maps, core_ids=list(range(N_CORES)), trace=trace,
        tmpdir=tmpdir)
    return _assemble(res.results), res


def kernel(**inputs):
    inputs = {k: np.asarray(v) for k, v in inputs.items()}
    out, _ = _run(inputs, trace=False)
    return out


# revision 14
# speedup vs baseline: 1.2347x; 1.0622x over previous
"""Vocab-sharded AdaptiveSoftmax (log_softmax loss head) on 8 TRN2 NeuronCores.

Reference, for x:[2,512,1024] (T=1024 tokens, H=1024):
  head  = x @ W_head.T          -> cols 0:20000 raw logits + 2 cluster logits
  tail1 = cl0 + log_softmax(x @ W_proj1.T @ W_tail1.T)   (40000 vocab)
  tail2 = cl1 + log_softmax(x @ W_proj2.T @ W_tail2.T)   (140000 vocab)
  out   = concat([head[:, :20000], tail1, tail2], -1)

Sharding: vocab dim of head/tail weights split 8 ways (2500/5000/17500 rows
per core, pre-transposed, x32-scaled + fp8e4-cast on host); x replicated.
log_softmax normalizers = AllReduce(add) of per-token exp-sums (the data
distribution keeps |logits| < ~3, so no max-subtraction is needed).

All matmuls run fp8e4 DoubleRow (2 contraction rows per pass); the 1/32
de-scale rides free on ACT `scale` / DVE tensor_scalar. Per 2048-col PSUM
seg: ACT computes exp+accum-sum from PSUM (discard output), and the seg
drains raw logits to a bf16 stage via DVE tensor_scalar_mul (most segs) or
ACT Identity (last 2-3 segs, balancing the engines). Finalize = one DVE
4x-mode tensor_scalar_add of (cl - ln S) over the whole stage. Only
Exp/Ln/Identity are used and the act-table chooser is pinned to the one
set containing all three, so the table loads once.

Phase order P -> T1 -> T2 -> H: tail1's two 4-tile AllReduces resolve
under tail2's first tiles, tail2's per-tile AllReduces resolve 2 tiles
later (finalize issued mid-tile t+2), and the PE-heavy head phase covers
tail2's last finalizes. A dummy warm-up AllReduce during P absorbs the
first-collective latency. Engine split: PE fp8-DR matmuls; DVE drains +
bias adds; ACT exps + a minority of drains; collectives on TOPSP; outputs
and weights on the Sync HWDGE queue, cc staging + g-loads on GpSimd's.
"""

import sys

import numpy as np

if "/opt/trn_rl_repo" not in sys.path:
    sys.path.insert(0, "/opt/trn_rl_repo")

P = 128
T = 1024          # tokens (2*512)
NT = T // P       # 8 token tiles
H = 1024
KO_H = H // P     # 8
VH = 2500         # head vocab shard
VHp = 2512        # padded to %16 for DoubleRow rhs step
V1 = 5000         # tail1 vocab shard
V1p = 5008
V2 = 17500        # tail2 vocab shard
V2p = 17504
E1, E2 = 512, 256
KO_1, KO_2 = E1 // P, E2 // P
C = 512           # matmul free-dim sub-block (psum bank / DR moving limit)
N_CORES = 8
VOUT = VH + V1 + V2   # 25000 per-core out cols
WSC = 32.0        # host-side weight scale into fp8 normal range
ISC = 1.0 / WSC
SEG = 2048        # psum tile width (4 f32 banks)

T2SEGS = [(i * SEG, SEG) for i in range(8)] + [(8 * SEG, V2p - 8 * SEG)]
T1SEGS = [(0, SEG), (SEG, SEG), (2 * SEG, V1p - 2 * SEG)]
HSEGS = [(0, SEG), (SEG, VHp - SEG)]

_CACHE = {}


def _pin_act_tables():
    """Make Exp/Ln resolve only to natural_log_exp_and_others so the ACT
    table loads once instead of thrashing between per-function sets.
    Set ids stay valid: we only shrink the fn sets used by the chooser."""
    import concourse.hw_specs as hw_specs
    import concourse.mybir as mybir
    tabs = hw_specs.get_activation_tables("gen3")  # functools.cached dict
    for name, fns in tabs.items():
        if name != "natural_log_exp_and_others":
            fns.discard(mybir.ActivationFunctionType.Exp)
            fns.discard(mybir.ActivationFunctionType.Ln)


def _build():
    import concourse.bacc as bacc
    import concourse.mybir as mybir
    import concourse.tile as tile
    from contextlib import ExitStack

    _pin_act_tables()

    f8 = mybir.dt.float8e4
    bf16 = mybir.dt.bfloat16
    f32 = mybir.dt.float32
    Exp = mybir.ActivationFunctionType.Exp
    Ident = mybir.ActivationFunctionType.Identity
    Ln = mybir.ActivationFunctionType.Ln
    DR = mybir.MatmulPerfMode.DoubleRow
    AX = mybir.AxisListType.X

    nc = bacc.Bacc("TRN2", target_bir_lowering=False, debug=False,
                   num_devices=N_CORES)

    xT_d = nc.declare_dram_parameter("xT", [P, KO_H, T], f8, False)
    whead_d = nc.declare_dram_parameter("wheadT", [P, KO_H, VHp], f8, False)
    wcl_d = nc.declare_dram_parameter("wclT", [P, KO_H, 2], f8, False)
    wp1_d = nc.declare_dram_parameter("wp1T", [P, KO_H, E1], f8, False)
    wp2_d = nc.declare_dram_parameter("wp2T", [P, KO_H, E2], f8, False)
    wt1_d = nc.declare_dram_parameter("wt1T", [P, KO_1, V1p], f8, False)
    wt2_d = nc.declare_dram_parameter("wt2T", [P, KO_2, V2p], f8, False)
    out_d = nc.declare_dram_parameter("out", [T, VOUT], bf16, True)

    out_r = out_d.ap().rearrange("(t p) v -> p t v", p=P)
    rg = [list(range(N_CORES))]

    with tile.TileContext(nc) as tc:
        with ExitStack() as root:
            pers = root.enter_context(tc.tile_pool(name="pers", bufs=1))
            psum = root.enter_context(
                tc.tile_pool(name="psum", bufs=2, space="PSUM"))
            dram = root.enter_context(
                tc.tile_pool(name="dram", bufs=1, space="DRAM"))
            scratch = root.enter_context(tc.tile_pool(name="scratch", bufs=2))

            # persistent small tiles
            p1T = pers.tile([P, KO_1, T], f8, name="p1T")
            p2T = pers.tile([P, KO_2, T], f8, name="p2T")
            cl = pers.tile([P, NT, 2], f32, name="cl")
            s1acc = pers.tile([P, NT, 3], f32, name="s1acc")
            s2acc = pers.tile([P, NT, 9], f32, name="s2acc")
            s1 = pers.tile([P, NT], f32, name="s1")
            s2 = pers.tile([P, NT], f32, name="s2")
            g1 = pers.tile([P, NT], f32, name="g1")
            g2 = pers.tile([P, NT], f32, name="g2")
            b1 = pers.tile([P, NT], f32, name="b1")
            b2 = pers.tile([P, NT], f32, name="b2")
            wrm = pers.tile([P, 1], f32, name="wrm")
            exb = scratch.tile([P, SEG], bf16, tag="exb", bufs=1)

            ccw_in = dram.tile([P, 1], f32, name="ccw_in")
            ccw_out = dram.tile([P, 1], f32, name="ccw_out",
                                addr_space="Shared")
            cc1_in = [dram.tile([P, 4], f32, name=f"cc1_in{i}")
                      for i in range(2)]
            cc1_out = [dram.tile([P, 4], f32, name=f"cc1_out{i}",
                                 addr_space="Shared") for i in range(2)]
            cc2_in = [dram.tile([P, 1], f32, name=f"cc2_in{t}")
                      for t in range(NT)]
            cc2_out = [dram.tile([P, 1], f32, name=f"cc2_out{t}",
                                 addr_space="Shared") for t in range(NT)]

            def mm_seg(ps, w, lhsT_sb, kop, t, rhs_sb, voff):
                """Accumulate [128 tokens, w] logits (x32 scale) into psum ps
                for token tile t via DoubleRow fp8: kop k-pairs, rhs columns
                voff:voff+w."""
                for kk in range(kop):
                    for sub in range(0, w, C):
                        sw = min(C, w - sub)
                        nc.tensor.matmul(
                            ps[:, sub:sub + sw],
                            lhsT_sb[:, 2 * kk:2 * kk + 2, t * P:(t + 1) * P],
                            rhs_sb[:, 2 * kk:2 * kk + 2,
                                   voff + sub:voff + sub + sw],
                            start=(kk == 0), stop=(kk == kop - 1),
                            perf_mode=DR)

            # ================= Phase P =================
            wp_pool = tc.alloc_tile_pool(name="wpp", bufs=1, side="right")
            wp1 = wp_pool.tile([P, KO_H, E1], f8, name="wp1")
            wp2 = wp_pool.tile([P, KO_H, E2], f8, name="wp2")
            wcl = wp_pool.tile([P, KO_H, 2], f8, name="wcl")
            xT_pool = tc.alloc_tile_pool(name="xTp", bufs=1)
            xT = xT_pool.tile([P, KO_H, T], f8, name="xT")
            wt1_pool = tc.alloc_tile_pool(name="wt1p", bufs=1)
            wt1 = wt1_pool.tile([P, KO_1, V1p], f8, name="wt1")

            nc.sync.dma_start(wp1[:], wp1_d[:])
            nc.sync.dma_start(wcl[:], wcl_d[:])
            nc.sync.dma_start(wp2[:], wp2_d[:])
            nc.sync.dma_start(xT[:], xT_d[:])
            nc.sync.dma_start(wt1[:], wt1_d[:])

            # warm up the collectives pipe (first AR pays ~30us extra)
            nc.vector.memset(wrm[:], 1.0)
            nc.gpsimd.dma_start(ccw_in[:], wrm[:])
            nc.gpsimd.collective_compute(
                "AllReduce", mybir.AluOpType.add, replica_groups=rg,
                ins=[ccw_in[:].opt()], outs=[ccw_out[:].opt()])

            for proj_sb, wp_sb, ko in ((p1T, wp1, KO_1), (p2T, wp2, KO_2)):
                for e in range(ko):
                    for th in range(2):
                        ps = psum.tile([P, SEG], f32, tag="mm")
                        for kk in range(KO_H // 2):
                            nc.tensor.matmul(
                                ps[:, :512],
                                wp_sb[:, 2 * kk:2 * kk + 2,
                                      e * P:(e + 1) * P],
                                xT[:, 2 * kk:2 * kk + 2,
                                   th * 512:(th + 1) * 512],
                                start=(kk == 0), stop=(kk == KO_H // 2 - 1),
                                perf_mode=DR)
                        nc.scalar.activation(
                            proj_sb[:, e, th * 512:(th + 1) * 512],
                            ps[:, :512], Ident, scale=ISC)
            for t in range(NT):
                ps = psum.tile([P, SEG], f32, tag="mm")
                for kk in range(KO_H // 2):
                    nc.tensor.matmul(
                        ps[:, :2],
                        xT[:, 2 * kk:2 * kk + 2, t * P:(t + 1) * P],
                        wcl[:, 2 * kk:2 * kk + 2, :],
                        start=(kk == 0), stop=(kk == KO_H // 2 - 1),
                        perf_mode=DR)
                nc.vector.tensor_scalar_mul(cl[:, t, :], ps[:, :2], ISC)
            wp_pool.release()

            wt2_pool = tc.alloc_tile_pool(name="wt2p", bufs=1, side="right")
            wt2 = wt2_pool.tile([P, KO_2, V2p], f8, name="wt2")
            for off, w in T2SEGS:
                nc.sync.dma_start(wt2[:, :, off:off + w],
                                  wt2_d[:, :, off:off + w])

            # ================= Phase T1 =================
            sp1 = tc.alloc_tile_pool(name="sp1", bufs=NT, side="right")
            stg1 = {}

            def t1_compute(t):
                stg = sp1.tile([P, V1], bf16, name=f"stg1_{t}", tag="s1")
                stg1[t] = stg
                for si, (off, w) in enumerate(T1SEGS):
                    ps = psum.tile([P, SEG], f32, tag="mm")
                    mm_seg(ps, w, p1T, KO_1 // 2, t, wt1, off)
                    wt = min(w, V1 - off)
                    nc.scalar.activation(
                        exb[:, :wt], ps[:, :wt], Exp, scale=ISC,
                        accum_out=s1acc[:, t, si:si + 1])
                    if si < 2:
                        nc.vector.tensor_scalar_mul(
                            stg[:, off:off + wt], ps[:, :wt], ISC)
                    else:
                        nc.scalar.activation(
                            stg[:, off:off + wt], ps[:, :wt], Ident,
                            scale=ISC)
                nc.vector.reduce_sum(s1[:, t:t + 1], s1acc[:, t, :], axis=AX)

            def t1_ar(i):  # i = batch 0 (tiles 0-3) or 1 (tiles 4-7)
                nc.gpsimd.dma_start(cc1_in[i][:], s1[:, 4 * i:4 * i + 4])
                nc.gpsimd.collective_compute(
                    "AllReduce", mybir.AluOpType.add, replica_groups=rg,
                    ins=[cc1_in[i][:].opt()], outs=[cc1_out[i][:].opt()])

            def t1_bias(i):
                nc.gpsimd.dma_start(g1[:, 4 * i:4 * i + 4], cc1_out[i][:])
                lng = scratch.tile([P, 4], f32, tag="lng4")
                nc.scalar.activation(lng[:], g1[:, 4 * i:4 * i + 4], Ln)
                nc.vector.tensor_sub(out=b1[:, 4 * i:4 * i + 4],
                                     in0=cl[:, 4 * i:4 * i + 4, 0],
                                     in1=lng[:])

            def t1_finalize(t):
                nc.vector.tensor_scalar_add(stg1[t][:], stg1[t][:],
                                            b1[:, t:t + 1])
                nc.sync.dma_start(out_r[:, t, VH:VH + V1], stg1[t][:])

            for t in range(4):
                t1_compute(t)
            t1_ar(0)
            for t in range(4, NT):
                t1_compute(t)
            t1_ar(1)
            wt1_pool.release()

            # ================= Phase T2 =================
            sp2a = tc.alloc_tile_pool(name="sp2a", bufs=2)
            sp2 = [sp2a, sp2a, None]
            stg2 = {}

            def t2_compute(t, mids=()):
                mids = list(mids)
                stg = sp2[t % 3].tile([P, V2], bf16, name=f"stg2_{t}",
                                      tag="s2")
                stg2[t] = stg
                nact = 3 if t % 2 else 2   # drains routed to ACT per tile
                for si, (off, w) in enumerate(T2SEGS):
                    if si == 2 and mids:
                        for m in mids:
                            m()
                    ps = psum.tile([P, SEG], f32, tag="mm")
                    mm_seg(ps, w, p2T, KO_2 // 2, t, wt2, off)
                    wt = min(w, V2 - off)
                    nc.scalar.activation(
                        exb[:, :wt], ps[:, :wt], Exp, scale=ISC,
                        accum_out=s2acc[:, t, si:si + 1])
                    if si < len(T2SEGS) - nact:
                        nc.vector.tensor_scalar_mul(
                            stg[:, off:off + wt], ps[:, :wt], ISC)
                    else:
                        nc.scalar.activation(
                            stg[:, off:off + wt], ps[:, :wt], Ident,
                            scale=ISC)
                nc.vector.reduce_sum(s2[:, t:t + 1], s2acc[:, t, :], axis=AX)
                nc.gpsimd.dma_start(cc2_in[t][:], s2[:, t:t + 1])
                nc.gpsimd.collective_compute(
                    "AllReduce", mybir.AluOpType.add, replica_groups=rg,
                    ins=[cc2_in[t][:].opt()], outs=[cc2_out[t][:].opt()])

            def t2_finalize(t):
                nc.gpsimd.dma_start(g2[:, t:t + 1], cc2_out[t][:])
                lng = scratch.tile([P, 1], f32, tag="lng")
                nc.scalar.activation(lng[:], g2[:, t:t + 1], Ln)
                nc.scalar.activation(b2[:, t:t + 1], lng[:], Ident,
                                     bias=cl[:, t, 1:2], scale=-1.0)
                nc.vector.tensor_scalar_add(stg2[t][:], stg2[t][:],
                                            b2[:, t:t + 1])
                nc.sync.dma_start(out_r[:, t, VH + V1:VOUT], stg2[t][:])

            t2_compute(0, mids=(lambda: t1_bias(0),
                                lambda: t1_finalize(0),
                                lambda: t1_finalize(1),
                                lambda: t1_finalize(2),
                                lambda: t1_finalize(3)))
            t2_compute(1, mids=(lambda: t1_bias(1),
                                lambda: t1_finalize(4),
                                lambda: t1_finalize(5),
                                lambda: t1_finalize(6),
                                lambda: t1_finalize(7)))
            sp1.release()
            sp2b = tc.alloc_tile_pool(name="sp2b", bufs=1)
            sp2[2] = sp2b
            whead_pool = tc.alloc_tile_pool(name="wheadp", bufs=1,
                                            side="right")
            whead = whead_pool.tile([P, KO_H, VHp], f8, name="whead")
            for t in range(2, NT):
                t2_compute(t, mids=(lambda tt=t: t2_finalize(tt - 2),))
                if t == 3:
                    nc.sync.dma_start(whead[:], whead_d[:])

            # ================= Phase H =================
            headout_pool = tc.alloc_tile_pool(name="headoutp", bufs=3,
                                              side="right")

            def h_tile(t):
                ho = headout_pool.tile([P, VHp], bf16, tag="ho")
                for si, (off, w) in enumerate(HSEGS):
                    ps = psum.tile([P, SEG], f32, tag="mm")
                    mm_seg(ps, w, xT, KO_H // 2, t, whead, off)
                    if si == 0:
                        nc.vector.tensor_scalar_mul(
                            ho[:, off:off + w], ps[:, :w], ISC)
                    else:
                        nc.scalar.activation(ho[:, off:off + w], ps[:, :w],
                                             Ident, scale=ISC)
                nc.sync.dma_start(out_r[:, t, 0:VH], ho[:, :VH])

            h_tile(0)
            t2_finalize(6)
            h_tile(1)
            h_tile(2)
            h_tile(3)
            t2_finalize(7)
            for t in range(4, NT):
                h_tile(t)

            # unwind pools (stack order per side)
            sp2b.release()
            sp2a.release()
            xT_pool.release()
            headout_pool.release()
            whead_pool.release()
            wt2_pool.release()

    nc.compile()
    return nc


def _get_nc():
    if "nc" not in _CACHE:
        _CACHE["nc"] = _build()
    return _CACHE["nc"]


def _prep_inputs(x, W_head, W_proj1, W_tail1, W_proj2, W_tail2):
    import concourse.mybir as mybir
    f8np = mybir.dt.np(mybir.dt.float8e4)

    def kxn8(w, pad_to=None, scale=WSC):
        # [N, K] weight -> [128, K//128, Np] fp8 (K on partitions), x scale
        n, k = w.shape
        a = np.ascontiguousarray(
            w.T.reshape(k // P, P, n).transpose(1, 0, 2)) * scale
        if pad_to is not None and pad_to != n:
            a = np.concatenate(
                [a, np.zeros((P, k // P, pad_to - n), np.float32)], axis=2)
        return a.astype(f8np)

    x2 = np.asarray(x, np.float32).reshape(T, H)
    xT = np.ascontiguousarray(
        x2.T.reshape(KO_H, P, T).transpose(1, 0, 2)).astype(f8np)
    wcl = kxn8(W_head[20000:20002])
    wp1 = kxn8(W_proj1)
    wp2 = kxn8(W_proj2)

    in_maps = []
    for i in range(N_CORES):
        in_maps.append({
            "xT": xT,
            "wheadT": kxn8(W_head[i * VH:(i + 1) * VH], VHp),
            "wclT": wcl,
            "wp1T": wp1,
            "wp2T": wp2,
            "wt1T": kxn8(W_tail1[i * V1:(i + 1) * V1], V1p),
            "wt2T": kxn8(W_tail2[i * V2:(i + 1) * V2], V2p),
        })
    return in_maps


def _assemble(outs):
    final = np.empty((T, 200000), dtype=np.float32)
    for i in range(N_CORES):
        o = np.asarray(outs[i]["out"])
        final[:, i * VH:(i + 1) * VH] = o[:, :VH]
        final[:, 20000 + i * V1:20000 + (i + 1) * V1] = o[:, VH:VH + V1]
        final[:, 60000 + i * V2:60000 + (i + 1) * V2] = o[:, VH + V1:]
    return final.reshape(2, 512, 200000)


def _run(inputs, trace=False, tmpdir=None):
    from concourse import bass_utils
    nc = _get_nc()
    in_maps = _prep_inputs(**inputs)
    res = bass_utils.run_bass_kernel_spmd(
        nc, in_maps, core_ids=list(range(N_CORES)), trace=trace,
        tmpdir=tmpdir)
    return _assemble(res.results), res


def kernel(**inputs):
    inputs = {k: np.asarray(v) for k, v in inputs.items()}
    out, _ = _run(inputs, trace=False)
    return out


# revision 22
# speedup vs baseline: 1.2711x; 1.0295x over previous
"""Vocab-sharded AdaptiveSoftmax (log_softmax loss head) on 8 TRN2 NeuronCores.

Reference, for x:[2,512,1024] (T=1024 tokens, H=1024):
  head  = x @ W_head.T          -> cols 0:20000 raw logits + 2 cluster logits
  tail1 = cl0 + log_softmax(x @ W_proj1.T @ W_tail1.T)   (40000 vocab)
  tail2 = cl1 + log_softmax(x @ W_proj2.T @ W_tail2.T)   (140000 vocab)
  out   = concat([head[:, :20000], tail1, tail2], -1)

Sharding: vocab dim of head/tail weights split 8 ways (2500/5000/17500 rows
per core, pre-transposed, x32-scaled + fp8e4-cast on host); x replicated.
log_softmax normalizers = AllReduce(add) of per-token exp-sums (the data
distribution keeps |logits| < ~3, so no max-subtraction is needed).

All matmuls run fp8e4 DoubleRow (2 contraction rows per pass); the 1/32
de-scale rides free on ACT `scale` / DVE tensor_scalar. Per 2048-col PSUM
seg: ACT computes exp+accum-sum from PSUM (discard output), and the seg
drains raw logits to a bf16 stage via DVE tensor_scalar_mul (most segs) or
ACT Identity (last 2-3 segs, balancing the engines). Finalize = one DVE
4x-mode tensor_scalar_add of (cl - ln S) over the whole stage. Only
Exp/Ln/Identity are used and the act-table chooser is pinned to the one
set containing all three, so the table loads once.

Phase order P -> T1 -> T2 -> H: tail1's two 4-tile AllReduces resolve
under tail2's first tiles, tail2's per-tile AllReduces resolve 2 tiles
later (finalize issued mid-tile t+2), and the PE-heavy head phase covers
tail2's last finalizes. A dummy warm-up AllReduce during P absorbs the
first-collective latency. Engine split: PE fp8-DR matmuls; DVE drains +
bias adds; ACT exps + a minority of drains; collectives on TOPSP; outputs
and weights on the Sync HWDGE queue, cc staging + g-loads on GpSimd's.
"""

import sys

import numpy as np

if "/opt/trn_rl_repo" not in sys.path:
    sys.path.insert(0, "/opt/trn_rl_repo")

P = 128
T = 1024          # tokens (2*512)
NT = T // P       # 8 token tiles
H = 1024
KO_H = H // P     # 8
VH = 2500         # head vocab shard
VHp = 2512        # padded to %16 for DoubleRow rhs step
V1 = 5000         # tail1 vocab shard
V1p = 5008
V2 = 17500        # tail2 vocab shard
V2p = 17504
E1, E2 = 512, 256
KO_1, KO_2 = E1 // P, E2 // P
C = 512           # matmul free-dim sub-block (psum bank / DR moving limit)
N_CORES = 8
VOUT = VH + V1 + V2   # 25000 per-core out cols
WSC = 32.0        # host-side weight scale into fp8 normal range
ISC = 1.0 / WSC
SEG = 2048        # psum tile width (4 f32 banks)

T2SEGS = [(i * SEG, SEG) for i in range(8)] + [(8 * SEG, V2p - 8 * SEG)]
T1SEGS = [(0, SEG), (SEG, SEG), (2 * SEG, V1p - 2 * SEG)]
HSEGS = [(0, SEG), (SEG, VHp - SEG)]
NA2 = 6                    # tail2 staged segs per tile; rest deferred
AW2 = NA2 * SEG            # 12288 staged cols
BW2 = V2 - AW2             # 5212 deferred cols (bias fused on recompute)

_CACHE = {}


def _pin_act_tables():
    """Make Exp/Ln resolve only to natural_log_exp_and_others so the ACT
    table loads once instead of thrashing between per-function sets.
    Set ids stay valid: we only shrink the fn sets used by the chooser."""
    import concourse.hw_specs as hw_specs
    import concourse.mybir as mybir
    tabs = hw_specs.get_activation_tables("gen3")  # functools.cached dict
    for name, fns in tabs.items():
        if name != "natural_log_exp_and_others":
            fns.discard(mybir.ActivationFunctionType.Exp)
            fns.discard(mybir.ActivationFunctionType.Ln)


def _build():
    import concourse.bacc as bacc
    import concourse.mybir as mybir
    import concourse.tile as tile
    from contextlib import ExitStack

    _pin_act_tables()

    f8 = mybir.dt.float8e4
    bf16 = mybir.dt.bfloat16
    f32 = mybir.dt.float32
    Exp = mybir.ActivationFunctionType.Exp
    Ident = mybir.ActivationFunctionType.Identity
    Ln = mybir.ActivationFunctionType.Ln
    DR = mybir.MatmulPerfMode.DoubleRow
    AX = mybir.AxisListType.X

    nc = bacc.Bacc("TRN2", target_bir_lowering=False, debug=False,
                   num_devices=N_CORES)

    xT_d = nc.declare_dram_parameter("xT", [P, KO_H, T], f8, False)
    whead_d = nc.declare_dram_parameter("wheadT", [P, KO_H, VHp], f8, False)
    wcl_d = nc.declare_dram_parameter("wclT", [P, KO_H, 2], f8, False)
    wp1_d = nc.declare_dram_parameter("wp1T", [P, KO_H, E1], f8, False)
    wp2_d = nc.declare_dram_parameter("wp2T", [P, KO_H, E2], f8, False)
    wt1_d = nc.declare_dram_parameter("wt1T", [P, KO_1, V1p], f8, False)
    wt2_d = nc.declare_dram_parameter("wt2T", [P, KO_2, V2p], f8, False)
    out_d = nc.declare_dram_parameter("out", [T, VOUT], bf16, True)

    out_r = out_d.ap().rearrange("(t p) v -> p t v", p=P)
    rg = [list(range(N_CORES))]

    with tile.TileContext(nc) as tc:
        with ExitStack() as root:
            pers = root.enter_context(tc.tile_pool(name="pers", bufs=1))
            psum = root.enter_context(
                tc.tile_pool(name="psum", bufs=2, space="PSUM"))
            dram = root.enter_context(
                tc.tile_pool(name="dram", bufs=1, space="DRAM"))
            scratch = root.enter_context(tc.tile_pool(name="scratch", bufs=2))

            # persistent small tiles
            p2T = pers.tile([P, KO_2, T], f8, name="p2T")
            cl = pers.tile([P, NT, 2], f32, name="cl")
            s1acc = pers.tile([P, NT, 3], f32, name="s1acc")
            s2acc = pers.tile([P, NT, 9], f32, name="s2acc")
            s1 = pers.tile([P, NT], f32, name="s1")
            s2 = pers.tile([P, NT], f32, name="s2")
            g1 = pers.tile([P, NT], f32, name="g1")
            g2 = pers.tile([P, NT], f32, name="g2")
            b1 = pers.tile([P, NT], f32, name="b1")
            b2 = pers.tile([P, NT], f32, name="b2")
            wrm = pers.tile([P, 1], f32, name="wrm")
            exb = scratch.tile([P, SEG], f8, tag="exb", bufs=1)

            ccw_in = dram.tile([P, 1], f32, name="ccw_in")
            ccw_out = dram.tile([P, 1], f32, name="ccw_out",
                                addr_space="Shared")
            cc1_in = [dram.tile([P, 4], f32, name=f"cc1_in{i}")
                      for i in range(2)]
            cc1_out = [dram.tile([P, 4], f32, name=f"cc1_out{i}",
                                 addr_space="Shared") for i in range(2)]
            cc2_in = [dram.tile([P, 2], f32, name=f"cc2_in{b}")
                      for b in range(NT // 2)]
            cc2_out = [dram.tile([P, 2], f32, name=f"cc2_out{b}",
                                 addr_space="Shared") for b in range(NT // 2)]

            def mm_seg(ps, w, lhsT_sb, kop, t, rhs_sb, voff):
                """Accumulate [128 tokens, w] logits (x32 scale) into psum ps
                for token tile t via DoubleRow fp8: kop k-pairs, rhs columns
                voff:voff+w."""
                for kk in range(kop):
                    for sub in range(0, w, C):
                        sw = min(C, w - sub)
                        nc.tensor.matmul(
                            ps[:, sub:sub + sw],
                            lhsT_sb[:, 2 * kk:2 * kk + 2, t * P:(t + 1) * P],
                            rhs_sb[:, 2 * kk:2 * kk + 2,
                                   voff + sub:voff + sub + sw],
                            start=(kk == 0), stop=(kk == kop - 1),
                            perf_mode=DR)

            # ================= Phase P =================
            xT_pool = tc.alloc_tile_pool(name="xTp", bufs=1, side="right")
            xT = xT_pool.tile([P, KO_H, T], f8, name="xT")
            whead_pool = tc.alloc_tile_pool(name="wheadp", bufs=1,
                                            side="right")
            whead = whead_pool.tile([P, KO_H, VHp], f8, name="whead")
            p1T_pool = tc.alloc_tile_pool(name="p1Tp", bufs=1)
            p1Tl = p1T_pool.tile([P, KO_1, T], f8, name="p1Tl")
            wt1_pool = tc.alloc_tile_pool(name="wt1p", bufs=1)
            wt1 = wt1_pool.tile([P, KO_1, V1p], f8, name="wt1")
            wp_pool = tc.alloc_tile_pool(name="wpp", bufs=1)
            wp1 = wp_pool.tile([P, KO_H, E1], f8, name="wp1")
            wp2 = wp_pool.tile([P, KO_H, E2], f8, name="wp2")
            wcl = wp_pool.tile([P, KO_H, 2], f8, name="wcl")

            nc.sync.dma_start(wp1[:], wp1_d[:])
            nc.sync.dma_start(wcl[:], wcl_d[:])
            nc.sync.dma_start(wp2[:], wp2_d[:])
            nc.sync.dma_start(xT[:], xT_d[:])
            nc.sync.dma_start(wt1[:], wt1_d[:])

            # warm up the collectives pipe (first AR pays ~30us extra)
            nc.vector.memset(wrm[:], 1.0)
            nc.gpsimd.dma_start(ccw_in[:], wrm[:])
            nc.gpsimd.collective_compute(
                "AllReduce", mybir.AluOpType.add, replica_groups=rg,
                ins=[ccw_in[:].opt()], outs=[ccw_out[:].opt()])

            for proj_sb, wp_sb, ko in ((p1Tl, wp1, KO_1), (p2T, wp2, KO_2)):
                for e in range(ko):
                    for th in range(2):
                        ps = psum.tile([P, SEG], f32, tag="mm")
                        for kk in range(KO_H // 2):
                            nc.tensor.matmul(
                                ps[:, :512],
                                wp_sb[:, 2 * kk:2 * kk + 2,
                                      e * P:(e + 1) * P],
                                xT[:, 2 * kk:2 * kk + 2,
                                   th * 512:(th + 1) * 512],
                                start=(kk == 0), stop=(kk == KO_H // 2 - 1),
                                perf_mode=DR)
                        nc.scalar.activation(
                            proj_sb[:, e, th * 512:(th + 1) * 512],
                            ps[:, :512], Ident, scale=ISC)
            for t in range(NT):
                ps = psum.tile([P, SEG], f32, tag="mm")
                for kk in range(KO_H // 2):
                    nc.tensor.matmul(
                        ps[:, :2],
                        xT[:, 2 * kk:2 * kk + 2, t * P:(t + 1) * P],
                        wcl[:, 2 * kk:2 * kk + 2, :],
                        start=(kk == 0), stop=(kk == KO_H // 2 - 1),
                        perf_mode=DR)
                nc.vector.tensor_scalar_mul(cl[:, t, :], ps[:, :2], ISC)
            wp_pool.release()

            wt2_pool = tc.alloc_tile_pool(name="wt2p", bufs=1, side="right")
            wt2 = wt2_pool.tile([P, KO_2, V2p], f8, name="wt2")
            for off, w in T2SEGS:
                nc.sync.dma_start(wt2[:, :, off:off + w],
                                  wt2_d[:, :, off:off + w])
            nc.sync.dma_start(whead[:], whead_d[:])

            # ================= Phase T1 =================
            # tiles 4-7 pool allocated first: it is released later (LIFO)
            sp1L = tc.alloc_tile_pool(name="sp1L", bufs=4, side="right")
            sp1E = tc.alloc_tile_pool(name="sp1E", bufs=4, side="right")
            stg1 = {}

            def t1_compute(t):
                pool = sp1E if t < 4 else sp1L
                stg = pool.tile([P, V1], bf16, name=f"stg1_{t}", tag="s1")
                stg1[t] = stg
                for si, (off, w) in enumerate(T1SEGS):
                    ps = psum.tile([P, SEG], f32, tag="mm")
                    mm_seg(ps, w, p1Tl, KO_1 // 2, t, wt1, off)
                    wt = min(w, V1 - off)
                    nc.scalar.activation(
                        exb[:, :wt], ps[:, :wt], Exp, scale=ISC,
                        accum_out=s1acc[:, t, si:si + 1])
                    if si < 2:
                        nc.vector.tensor_scalar_mul(
                            stg[:, off:off + wt], ps[:, :wt], ISC)
                    else:
                        nc.scalar.activation(
                            stg[:, off:off + wt], ps[:, :wt], Ident,
                            scale=ISC)
                nc.vector.reduce_sum(s1[:, t:t + 1], s1acc[:, t, :], axis=AX)

            def t1_ar(i):  # i = batch 0 (tiles 0-3) or 1 (tiles 4-7)
                nc.gpsimd.dma_start(cc1_in[i][:], s1[:, 4 * i:4 * i + 4])
                nc.gpsimd.collective_compute(
                    "AllReduce", mybir.AluOpType.add, replica_groups=rg,
                    ins=[cc1_in[i][:].opt()], outs=[cc1_out[i][:].opt()])

            def t1_bias(i):
                nc.gpsimd.dma_start(g1[:, 4 * i:4 * i + 4], cc1_out[i][:])
                lng = scratch.tile([P, 4], f32, tag="lng4")
                nc.scalar.activation(lng[:], g1[:, 4 * i:4 * i + 4], Ln)
                nc.vector.tensor_sub(out=b1[:, 4 * i:4 * i + 4],
                                     in0=cl[:, 4 * i:4 * i + 4, 0],
                                     in1=lng[:])

            def t1_finalize(t):
                nc.vector.tensor_scalar_add(stg1[t][:], stg1[t][:],
                                            b1[:, t:t + 1])
                nc.sync.dma_start(out_r[:, t, VH:VH + V1], stg1[t][:])

            for t in range(4):
                t1_compute(t)
            t1_ar(0)
            t1_compute(4)
            t1_compute(5)
            t1_compute(6)
            t1_bias(0)
            t1_finalize(0)
            t1_finalize(1)
            t1_compute(7)
            t1_ar(1)
            t1_finalize(2)
            t1_finalize(3)
            sp1E.release()
            wt1_pool.release()
            p1T_pool.release()

            # ================= Phase T2 (head tiles interleaved) ========
            sp2a = tc.alloc_tile_pool(name="sp2a", bufs=2)
            sp2b = [None]   # allocated after sp1L release
            obuf_pool = [None]
            stg2 = {}
            DEFSEGS = T2SEGS[NA2:]

            def t2_compute(t, mids=(), misi=2):
                mids = list(mids)
                pool = sp2a if (t // 2) % 2 == 0 else sp2b[0]
                stg = pool.tile([P, AW2], bf16, name=f"stg2_{t}", tag="s2")
                stg2[t] = stg
                for si, (off, w) in enumerate(T2SEGS):
                    if si == misi and mids:
                        for m in mids:
                            m()
                    ps = psum.tile([P, SEG], f32, tag="mm")
                    mm_seg(ps, w, p2T, KO_2 // 2, t, wt2, off)
                    wt = min(w, V2 - off)
                    nc.scalar.activation(
                        exb[:, :wt], ps[:, :wt], Exp, scale=ISC,
                        accum_out=s2acc[:, t, si:si + 1])
                    if si < NA2:
                        nc.vector.tensor_scalar_mul(
                            stg[:, off:off + wt], ps[:, :wt], ISC)
                nc.vector.reduce_sum(s2[:, t:t + 1], s2acc[:, t, :], axis=AX)
                if t % 2:
                    b = t // 2
                    nc.gpsimd.dma_start(cc2_in[b][:], s2[:, t - 1:t + 1])
                    nc.gpsimd.collective_compute(
                        "AllReduce", mybir.AluOpType.add, replica_groups=rg,
                        ins=[cc2_in[b][:].opt()], outs=[cc2_out[b][:].opt()])

            def t2_bias(b):
                nc.gpsimd.dma_start(g2[:, 2 * b:2 * b + 2], cc2_out[b][:])
                lng = scratch.tile([P, 2], f32, tag="lng2")
                nc.scalar.activation(lng[:], g2[:, 2 * b:2 * b + 2], Ln)
                nc.vector.tensor_sub(out=b2[:, 2 * b:2 * b + 2],
                                     in0=cl[:, 2 * b:2 * b + 2, 1],
                                     in1=lng[:])

            def t2_finalize(t):
                # staged region: add bias, ship
                nc.vector.tensor_scalar_add(stg2[t][:], stg2[t][:],
                                            b2[:, t:t + 1])
                nc.sync.dma_start(out_r[:, t, VH + V1:VH + V1 + AW2],
                                  stg2[t][:])
                # deferred region: recompute, drain with fused bias
                ob = obuf_pool[0].tile([P, BW2], bf16, tag="ob")
                for si, (off, w) in enumerate(DEFSEGS):
                    ps = psum.tile([P, SEG], f32, tag="mm")
                    mm_seg(ps, w, p2T, KO_2 // 2, t, wt2, off)
                    wt = min(w, V2 - off)
                    o = ob[:, off - AW2:off - AW2 + wt]
                    if si < len(DEFSEGS) - 1:
                        nc.scalar.activation(o, ps[:, :wt], Ident,
                                             scale=ISC, bias=b2[:, t:t + 1])
                    else:
                        nc.vector.tensor_scalar(
                            o, ps[:, :wt], ISC, b2[:, t:t + 1],
                            mybir.AluOpType.mult, mybir.AluOpType.add)
                nc.sync.dma_start(out_r[:, t, VH + V1 + AW2:VOUT],
                                  ob[:, :BW2])

            # ================= Phase H (interleaved) =================
            def h_tile(t):
                ho = headout_pool.tile([P, VHp], bf16, tag="ho")
                for si, (off, w) in enumerate(HSEGS):
                    ps = psum.tile([P, SEG], f32, tag="mm")
                    mm_seg(ps, w, xT, KO_H // 2, t, whead, off)
                    if si == 0:
                        nc.vector.tensor_scalar_mul(
                            ho[:, off:off + w], ps[:, :w], ISC)
                    else:
                        nc.scalar.activation(ho[:, off:off + w], ps[:, :w],
                                             Ident, scale=ISC)
                nc.sync.dma_start(out_r[:, t, 0:VH], ho[:, :VH])

            t2_compute(0, mids=(lambda: t1_bias(1),
                                lambda: t1_finalize(4),
                                lambda: t1_finalize(5),
                                lambda: t1_finalize(6),
                                lambda: t1_finalize(7)), misi=5)
            sp1L.release()
            sp2b[0] = tc.alloc_tile_pool(name="sp2b", bufs=2)
            obuf_pool[0] = tc.alloc_tile_pool(name="obuf", bufs=2,
                                              side="right")
            headout_pool = tc.alloc_tile_pool(name="headoutp", bufs=3,
                                              side="right")
            t2_compute(1)
            t2_compute(2)
            h_tile(0)
            t2_compute(3, mids=(lambda: t2_bias(0),
                                lambda: t2_finalize(0),
                                lambda: t2_finalize(1)))
            h_tile(1)
            t2_compute(4)
            h_tile(2)
            t2_compute(5, mids=(lambda: t2_bias(1),
                                lambda: t2_finalize(2),
                                lambda: t2_finalize(3)))
            h_tile(3)
            t2_compute(6)
            h_tile(4)
            t2_compute(7, mids=(lambda: t2_bias(2),
                                lambda: t2_finalize(4),
                                lambda: t2_finalize(5)))
            h_tile(5)
            h_tile(6)
            t2_bias(3)
            t2_finalize(6)
            h_tile(7)
            t2_finalize(7)

            # unwind pools (stack order per side)
            sp2b[0].release()
            sp2a.release()
            headout_pool.release()
            obuf_pool[0].release()
            wt2_pool.release()
            whead_pool.release()
            xT_pool.release()

    nc.compile()
    return nc


def _get_nc():
    if "nc" not in _CACHE:
        _CACHE["nc"] = _build()
    return _CACHE["nc"]


def _prep_inputs(x, W_head, W_proj1, W_tail1, W_proj2, W_tail2):
    import concourse.mybir as mybir
    f8np = mybir.dt.np(mybir.dt.float8e4)

    def kxn8(w, pad_to=None, scale=WSC):
        # [N, K] weight -> [128, K//128, Np] fp8 (K on partitions), x scale
        n, k = w.shape
        a = np.ascontiguousarray(
            w.T.reshape(k // P, P, n).transpose(1, 0, 2)) * scale
        if pad_to is not None and pad_to != n:
            a = np.concatenate(
                [a, np.zeros((P, k // P, pad_to - n), np.float32)], axis=2)
        return a.astype(f8np)

    x2 = np.asarray(x, np.float32).reshape(T, H)
    xT = np.ascontiguousarray(
        x2.T.reshape(KO_H, P, T).transpose(1, 0, 2)).astype(f8np)
    wcl = kxn8(W_head[20000:20002])
    wp1 = kxn8(W_proj1)
    wp2 = kxn8(W_proj2)

    in_maps = []
    for i in range(N_CORES):
        in_maps.append({
            "xT": xT,
            "wheadT": kxn8(W_head[i * VH:(i + 1) * VH], VHp),
            "wclT": wcl,
            "wp1T": wp1,
            "wp2T": wp2,
            "wt1T": kxn8(W_tail1[i * V1:(i + 1) * V1], V1p),
            "wt2T": kxn8(W_tail2[i * V2:(i + 1) * V2], V2p),
        })
    return in_maps


def _assemble(outs):
    final = np.empty((T, 200000), dtype=np.float32)
    for i in range(N_CORES):
        o = np.asarray(outs[i]["out"])
        final[:, i * VH:(i + 1) * VH] = o[:, :VH]
        final[:, 20000 + i * V1:20000 + (i + 1) * V1] = o[:, VH:VH + V1]
        final[:, 60000 + i * V2:60000 + (i + 1) * V2] = o[:, VH + V1:]
    return final.reshape(2, 512, 200000)


def _run(inputs, trace=False, tmpdir=None):
    from concourse import bass_utils
    nc = _get_nc()
    in_maps = _prep_inputs(**inputs)
    res = bass_utils.run_bass_kernel_spmd(
        nc, in_maps, core_ids=list(range(N_CORES)), trace=trace,
        tmpdir=tmpdir)
    return _assemble(res.results), res


def kernel(**inputs):
    inputs = {k: np.asarray(v) for k, v in inputs.items()}
    out, _ = _run(inputs, trace=False)
    return out


# revision 29
# speedup vs baseline: 1.2908x; 1.0155x over previous
"""Vocab-sharded AdaptiveSoftmax (log_softmax loss head) on 8 TRN2 NeuronCores.

Reference, for x:[2,512,1024] (T=1024 tokens, H=1024):
  head  = x @ W_head.T          -> cols 0:20000 raw logits + 2 cluster logits
  tail1 = cl0 + log_softmax(x @ W_proj1.T @ W_tail1.T)   (40000 vocab)
  tail2 = cl1 + log_softmax(x @ W_proj2.T @ W_tail2.T)   (140000 vocab)
  out   = concat([head[:, :20000], tail1, tail2], -1)

Sharding: vocab dim of head/tail weights split 8 ways (2500/5000/17500 rows
per core, pre-transposed, x32-scaled + fp8e4-cast on host); x replicated.
log_softmax normalizers = AllReduce(add) of per-token exp-sums (the data
distribution keeps |logits| < ~3, so no max-subtraction is needed).

All matmuls run fp8e4 DoubleRow (2 contraction rows per pass); the 1/32
de-scale rides free on ACT `scale` / DVE tensor_scalar. Per 2048-col PSUM
seg: ACT computes exp+accum-sum from PSUM (discard output), and the seg
drains raw logits to a bf16 stage via DVE tensor_scalar_mul (most segs) or
ACT Identity (last 2-3 segs, balancing the engines). Finalize = one DVE
4x-mode tensor_scalar_add of (cl - ln S) over the whole stage. Only
Exp/Ln/Identity are used and the act-table chooser is pinned to the one
set containing all three, so the table loads once.

Phase order P -> T1 -> T2 -> H: tail1's two 4-tile AllReduces resolve
under tail2's first tiles, tail2's per-tile AllReduces resolve 2 tiles
later (finalize issued mid-tile t+2), and the PE-heavy head phase covers
tail2's last finalizes. A dummy warm-up AllReduce during P absorbs the
first-collective latency. Engine split: PE fp8-DR matmuls; DVE drains +
bias adds; ACT exps + a minority of drains; collectives on TOPSP; outputs
and weights on the Sync HWDGE queue, cc staging + g-loads on GpSimd's.
"""

import sys

import numpy as np

if "/opt/trn_rl_repo" not in sys.path:
    sys.path.insert(0, "/opt/trn_rl_repo")

P = 128
T = 1024          # tokens (2*512)
NT = T // P       # 8 token tiles
H = 1024
KO_H = H // P     # 8
VH = 2500         # head vocab shard
VHp = 2512        # padded to %16 for DoubleRow rhs step
V1 = 5000         # tail1 vocab shard
V1p = 5008
V2 = 17500        # tail2 vocab shard
V2p = 17504
E1, E2 = 512, 256
KO_1, KO_2 = E1 // P, E2 // P
C = 512           # matmul free-dim sub-block (psum bank / DR moving limit)
N_CORES = 8
VOUT = VH + V1 + V2   # 25000 per-core out cols
WSC = 32.0        # host-side weight scale into fp8 normal range
ISC = 1.0 / WSC
SEG = 1536        # tail psum tile width (3 f32 banks)

T2SEGS = [(i * SEG, SEG) for i in range(11)] + [(11 * SEG, V2p - 11 * SEG)]
T1SEGS = [(i * SEG, SEG) for i in range(3)] + [(3 * SEG, V1p - 3 * SEG)]
HSUBS = [(0, 512), (512, 512), (1024, 512), (1536, 512), (2048, VHp - 2048)]
NA2 = 8                    # tail2 staged segs per tile; rest deferred
AW2 = NA2 * SEG            # 12288 staged cols
BW2 = V2 - AW2             # 5212 deferred cols (bias fused on recompute)
DEFSEGS = T2SEGS[NA2:]

_CACHE = {}


def _pin_act_tables():
    """Make Exp/Ln resolve only to natural_log_exp_and_others so the ACT
    table loads once instead of thrashing between per-function sets.
    Set ids stay valid: we only shrink the fn sets used by the chooser."""
    import concourse.hw_specs as hw_specs
    import concourse.mybir as mybir
    tabs = hw_specs.get_activation_tables("gen3")  # functools.cached dict
    for name, fns in tabs.items():
        if name != "natural_log_exp_and_others":
            fns.discard(mybir.ActivationFunctionType.Exp)
            fns.discard(mybir.ActivationFunctionType.Ln)


def _build():
    import concourse.bacc as bacc
    import concourse.mybir as mybir
    import concourse.tile as tile
    from contextlib import ExitStack

    _pin_act_tables()

    f8 = mybir.dt.float8e4
    bf16 = mybir.dt.bfloat16
    f32 = mybir.dt.float32
    Exp = mybir.ActivationFunctionType.Exp
    Ident = mybir.ActivationFunctionType.Identity
    Ln = mybir.ActivationFunctionType.Ln
    DR = mybir.MatmulPerfMode.DoubleRow
    AX = mybir.AxisListType.X

    nc = bacc.Bacc("TRN2", target_bir_lowering=False, debug=False,
                   num_devices=N_CORES)

    xT_d = nc.declare_dram_parameter("xT", [P, KO_H, T], f8, False)
    whead_d = nc.declare_dram_parameter("wheadT", [P, KO_H, VHp], f8, False)
    wcl_d = nc.declare_dram_parameter("wclT", [P, KO_H, 2], f8, False)
    wp1_d = nc.declare_dram_parameter("wp1T", [P, KO_H, E1], f8, False)
    wp2_d = nc.declare_dram_parameter("wp2T", [P, KO_H, E2], f8, False)
    wt1_d = nc.declare_dram_parameter("wt1T", [P, KO_1, V1p], f8, False)
    wt2_d = nc.declare_dram_parameter("wt2T", [P, KO_2, V2p], f8, False)
    out_d = nc.declare_dram_parameter("out", [T, VOUT], bf16, True)

    out_r = out_d.ap().rearrange("(t p) v -> p t v", p=P)
    rg = [list(range(N_CORES))]

    with tile.TileContext(nc) as tc:
        with ExitStack() as root:
            pers = root.enter_context(tc.tile_pool(name="pers", bufs=1))
            psum = root.enter_context(
                tc.tile_pool(name="psum", bufs=2, space="PSUM"))
            psumS = root.enter_context(
                tc.tile_pool(name="psumS", bufs=2, space="PSUM"))
            dram = root.enter_context(
                tc.tile_pool(name="dram", bufs=1, space="DRAM"))
            scratch = root.enter_context(tc.tile_pool(name="scratch", bufs=2))

            # persistent small tiles
            p2T = pers.tile([P, KO_2, T], f8, name="p2T")
            cl = pers.tile([P, NT, 2], f32, name="cl")
            s1acc = pers.tile([P, NT, 4], f32, name="s1acc")
            s2acc = pers.tile([P, NT, 12], f32, name="s2acc")
            s1 = pers.tile([P, NT], f32, name="s1")
            s2 = pers.tile([P, NT], f32, name="s2")
            g1 = pers.tile([P, NT], f32, name="g1")
            g2 = pers.tile([P, NT], f32, name="g2")
            b1 = pers.tile([P, NT], f32, name="b1")
            b2 = pers.tile([P, NT], f32, name="b2")
            wrm = pers.tile([P, 1], f32, name="wrm")
            exb = scratch.tile([P, SEG], f8, tag="exb", bufs=1)

            ccw_in = dram.tile([P, 1], f32, name="ccw_in")
            ccw_out = dram.tile([P, 1], f32, name="ccw_out",
                                addr_space="Shared")
            cc1_in = [dram.tile([P, 4], f32, name=f"cc1_in{i}")
                      for i in range(2)]
            cc1_out = [dram.tile([P, 4], f32, name=f"cc1_out{i}",
                                 addr_space="Shared") for i in range(2)]
            cc2_in = [dram.tile([P, 2], f32, name=f"cc2_in{b}")
                      for b in range(NT // 2)]
            cc2_out = [dram.tile([P, 2], f32, name=f"cc2_out{b}",
                                 addr_space="Shared") for b in range(NT // 2)]

            def mm_seg(ps, w, lhsT_sb, kop, t, rhs_sb, voff):
                """Accumulate [128 tokens, w] logits (x32 scale) into psum ps
                for token tile t via DoubleRow fp8: kop k-pairs, rhs columns
                voff:voff+w."""
                for kk in range(kop):
                    for sub in range(0, w, C):
                        sw = min(C, w - sub)
                        nc.tensor.matmul(
                            ps[:, sub:sub + sw],
                            lhsT_sb[:, 2 * kk:2 * kk + 2, t * P:(t + 1) * P],
                            rhs_sb[:, 2 * kk:2 * kk + 2,
                                   voff + sub:voff + sub + sw],
                            start=(kk == 0), stop=(kk == kop - 1),
                            perf_mode=DR)

            # ================= Phase P =================
            xT_pool = tc.alloc_tile_pool(name="xTp", bufs=1, side="right")
            xT = xT_pool.tile([P, KO_H, T], f8, name="xT")
            whead_pool = tc.alloc_tile_pool(name="wheadp", bufs=1,
                                            side="right")
            whead = whead_pool.tile([P, KO_H, VHp], f8, name="whead")
            p1T_pool = tc.alloc_tile_pool(name="p1Tp", bufs=1)
            p1Tl = p1T_pool.tile([P, KO_1, T], f8, name="p1Tl")
            wt1_pool = tc.alloc_tile_pool(name="wt1p", bufs=1)
            wt1 = wt1_pool.tile([P, KO_1, V1p], f8, name="wt1")
            wp_pool = tc.alloc_tile_pool(name="wpp", bufs=1)
            wp1 = wp_pool.tile([P, KO_H, E1], f8, name="wp1")
            wp2 = wp_pool.tile([P, KO_H, E2], f8, name="wp2")
            wcl = wp_pool.tile([P, KO_H, 2], f8, name="wcl")

            nc.sync.dma_start(wp1[:], wp1_d[:])
            nc.sync.dma_start(wcl[:], wcl_d[:])
            nc.sync.dma_start(wp2[:], wp2_d[:])
            nc.sync.dma_start(xT[:], xT_d[:])
            nc.sync.dma_start(wt1[:], wt1_d[:])

            # warm up the collectives pipe (first AR pays ~30us extra)
            nc.vector.memset(wrm[:], 1.0)
            nc.gpsimd.dma_start(ccw_in[:], wrm[:])
            nc.gpsimd.collective_compute(
                "AllReduce", mybir.AluOpType.add, replica_groups=rg,
                ins=[ccw_in[:].opt()], outs=[ccw_out[:].opt()])

            for proj_sb, wp_sb, ko in ((p1Tl, wp1, KO_1), (p2T, wp2, KO_2)):
                for e in range(ko):
                    for th in range(2):
                        ps = psumS.tile([P, 512], f32, tag="hs")
                        for kk in range(KO_H // 2):
                            nc.tensor.matmul(
                                ps[:],
                                wp_sb[:, 2 * kk:2 * kk + 2,
                                      e * P:(e + 1) * P],
                                xT[:, 2 * kk:2 * kk + 2,
                                   th * 512:(th + 1) * 512],
                                start=(kk == 0), stop=(kk == KO_H // 2 - 1),
                                perf_mode=DR)
                        nc.scalar.activation(
                            proj_sb[:, e, th * 512:(th + 1) * 512],
                            ps[:], Ident, scale=ISC)
            for t in range(NT):
                ps = psumS.tile([P, 512], f32, tag="hs")
                for kk in range(KO_H // 2):
                    nc.tensor.matmul(
                        ps[:, :2],
                        xT[:, 2 * kk:2 * kk + 2, t * P:(t + 1) * P],
                        wcl[:, 2 * kk:2 * kk + 2, :],
                        start=(kk == 0), stop=(kk == KO_H // 2 - 1),
                        perf_mode=DR)
                nc.vector.tensor_scalar_mul(cl[:, t, :], ps[:, :2], ISC)
            wp_pool.release()

            wt2_pool = tc.alloc_tile_pool(name="wt2p", bufs=1, side="right")
            wt2 = wt2_pool.tile([P, KO_2, V2p], f8, name="wt2")
            for off, w in T2SEGS:
                nc.sync.dma_start(wt2[:, :, off:off + w],
                                  wt2_d[:, :, off:off + w])
            nc.sync.dma_start(whead[:], whead_d[:])

            # ================= Phase T1 =================
            # tiles 4-7 pool allocated first: it is released later (LIFO)
            sp1L = tc.alloc_tile_pool(name="sp1L", bufs=4, side="right")
            sp1E = tc.alloc_tile_pool(name="sp1E", bufs=4, side="right")
            stg1 = {}

            def t1_compute(t):
                pool = sp1E if t < 4 else sp1L
                stg = pool.tile([P, V1], bf16, name=f"stg1_{t}", tag="s1")
                stg1[t] = stg
                for si, (off, w) in enumerate(T1SEGS):
                    if w > 512:
                        ps = psum.tile([P, SEG], f32, tag="mm")
                    else:
                        ps = psumS.tile([P, 512], f32, tag="hs")
                    mm_seg(ps, w, p1Tl, KO_1 // 2, t, wt1, off)
                    wt = min(w, V1 - off)
                    nc.scalar.activation(
                        exb[:, :wt], ps[:, :wt], Exp, scale=ISC,
                        accum_out=s1acc[:, t, si:si + 1])
                    if si < 3:
                        nc.vector.tensor_scalar_mul(
                            stg[:, off:off + wt], ps[:, :wt], ISC)
                    else:
                        nc.scalar.activation(
                            stg[:, off:off + wt], ps[:, :wt], Ident,
                            scale=ISC)
                nc.vector.reduce_sum(s1[:, t:t + 1], s1acc[:, t, :], axis=AX)

            def t1_ar(i):  # i = batch 0 (tiles 0-3) or 1 (tiles 4-7)
                nc.gpsimd.dma_start(cc1_in[i][:], s1[:, 4 * i:4 * i + 4])
                nc.gpsimd.collective_compute(
                    "AllReduce", mybir.AluOpType.add, replica_groups=rg,
                    ins=[cc1_in[i][:].opt()], outs=[cc1_out[i][:].opt()])

            def t1_bias(i):
                nc.gpsimd.dma_start(g1[:, 4 * i:4 * i + 4], cc1_out[i][:])
                lng = scratch.tile([P, 4], f32, tag="lng4")
                nc.scalar.activation(lng[:], g1[:, 4 * i:4 * i + 4], Ln)
                nc.vector.tensor_sub(out=b1[:, 4 * i:4 * i + 4],
                                     in0=cl[:, 4 * i:4 * i + 4, 0],
                                     in1=lng[:])

            def t1_finalize(t):
                nc.vector.tensor_scalar_add(stg1[t][:], stg1[t][:],
                                            b1[:, t:t + 1])
                nc.sync.dma_start(out_r[:, t, VH:VH + V1], stg1[t][:])

            for t in range(4):
                t1_compute(t)
            t1_ar(0)
            t1_compute(4)
            t1_compute(5)
            t1_compute(6)
            t1_bias(0)
            t1_finalize(0)
            t1_finalize(1)
            t1_compute(7)
            t1_ar(1)
            t1_finalize(2)
            t1_finalize(3)
            sp1E.release()
            wt1_pool.release()
            p1T_pool.release()

            # ================= Phase T2 (head tiles interleaved) ========
            headout_pool = tc.alloc_tile_pool(name="headoutp", bufs=3)
            sp2a = tc.alloc_tile_pool(name="sp2a", bufs=2)
            sp2b = [None]   # allocated after sp1L release
            obuf_pool = [None]
            stg2 = {}

            def t2_compute(t, mids=()):
                mids = dict(mids)
                pool = sp2a if (t // 2) % 2 == 0 else sp2b[0]
                stg = pool.tile([P, AW2], bf16, name=f"stg2_{t}", tag="s2")
                stg2[t] = stg
                for si, (off, w) in enumerate(T2SEGS):
                    for m in mids.get(si, ()):
                        m()
                    ps = psum.tile([P, SEG], f32, tag="mm")
                    mm_seg(ps, w, p2T, KO_2 // 2, t, wt2, off)
                    wt = min(w, V2 - off)
                    nc.scalar.activation(
                        exb[:, :wt], ps[:, :wt], Exp, scale=ISC,
                        accum_out=s2acc[:, t, si:si + 1])
                    if si < NA2:
                        nc.vector.tensor_scalar_mul(
                            stg[:, off:off + wt], ps[:, :wt], ISC)
                nc.vector.reduce_sum(s2[:, t:t + 1], s2acc[:, t, :], axis=AX)
                if t % 2:
                    b = t // 2
                    nc.gpsimd.dma_start(cc2_in[b][:], s2[:, t - 1:t + 1])
                    nc.gpsimd.collective_compute(
                        "AllReduce", mybir.AluOpType.add, replica_groups=rg,
                        ins=[cc2_in[b][:].opt()], outs=[cc2_out[b][:].opt()])

            def t2_bias(b):
                nc.gpsimd.dma_start(g2[:, 2 * b:2 * b + 2], cc2_out[b][:])
                lng = scratch.tile([P, 2], f32, tag="lng2")
                nc.scalar.activation(lng[:], g2[:, 2 * b:2 * b + 2], Ln)
                nc.vector.tensor_sub(out=b2[:, 2 * b:2 * b + 2],
                                     in0=cl[:, 2 * b:2 * b + 2, 1],
                                     in1=lng[:])

            def t2_finA(t):
                # staged region: add bias, ship
                nc.vector.tensor_scalar_add(stg2[t][:], stg2[t][:],
                                            b2[:, t:t + 1])
                nc.sync.dma_start(out_r[:, t, VH + V1:VH + V1 + AW2],
                                  stg2[t][:])

            def t2_def(t):
                # deferred region: recompute, drain with fused bias
                ob = obuf_pool[0].tile([P, BW2], bf16, tag="ob")
                for si, (off, w) in enumerate(DEFSEGS):
                    ps = psum.tile([P, SEG], f32, tag="mm")
                    mm_seg(ps, w, p2T, KO_2 // 2, t, wt2, off)
                    wt = min(w, V2 - off)
                    o = ob[:, off - AW2:off - AW2 + wt]
                    if si < 2:
                        nc.scalar.activation(o, ps[:, :wt], Ident,
                                             scale=ISC, bias=b2[:, t:t + 1])
                    else:
                        nc.vector.tensor_scalar(
                            o, ps[:, :wt], ISC, b2[:, t:t + 1],
                            mybir.AluOpType.mult, mybir.AluOpType.add)
                nc.sync.dma_start(out_r[:, t, VH + V1 + AW2:VOUT],
                                  ob[:, :BW2])

            # ============ Phase H (fine-grained interleave) ============
            # head subs run on their own 1-bank psum stream so the PE
            # always has an independent matmul to fill drain-bound gaps.
            def h_tile(t):
                ho = headout_pool.tile([P, VHp], bf16, tag="ho")
                for si, (off, w) in enumerate(HSUBS):
                    ps = psumS.tile([P, 512], f32, tag="hs")
                    for kk in range(KO_H // 2):
                        nc.tensor.matmul(
                            ps[:, :w],
                            xT[:, 2 * kk:2 * kk + 2, t * P:(t + 1) * P],
                            whead[:, 2 * kk:2 * kk + 2, off:off + w],
                            start=(kk == 0), stop=(kk == KO_H // 2 - 1),
                            perf_mode=DR)
                    if si < 3:
                        nc.vector.tensor_scalar_mul(
                            ho[:, off:off + w], ps[:, :w], ISC)
                    else:
                        nc.scalar.activation(ho[:, off:off + w], ps[:, :w],
                                             Ident, scale=ISC)
                nc.sync.dma_start(out_r[:, t, 0:VH], ho[:, :VH])

            h_tile(0)
            t2_compute(0, mids={5: (lambda: t1_bias(1),
                                    lambda: t1_finalize(4),
                                    lambda: t1_finalize(5),
                                    lambda: t1_finalize(6),
                                    lambda: t1_finalize(7))})
            sp1L.release()
            sp2b[0] = tc.alloc_tile_pool(name="sp2b", bufs=2)
            obuf_pool[0] = tc.alloc_tile_pool(name="obuf", bufs=2,
                                              side="right")
            t2_compute(1)
            h_tile(1)
            t2_compute(2)
            h_tile(2)
            t2_compute(3, mids={3: (lambda: t2_bias(0),
                                    lambda: t2_finA(0)),
                                6: (lambda: t2_def(0),
                                    lambda: t2_finA(1)),
                                9: (lambda: t2_def(1),)})
            h_tile(3)
            t2_compute(4)
            h_tile(4)
            t2_compute(5, mids={3: (lambda: t2_bias(1),
                                    lambda: t2_finA(2)),
                                6: (lambda: t2_def(2),
                                    lambda: t2_finA(3)),
                                9: (lambda: t2_def(3),)})
            h_tile(5)
            t2_compute(6)
            t2_compute(7, mids={3: (lambda: t2_bias(2),
                                    lambda: t2_finA(4)),
                                6: (lambda: t2_def(4),
                                    lambda: t2_finA(5)),
                                9: (lambda: t2_def(5),)})
            h_tile(6)
            h_tile(7)
            t2_bias(3)
            t2_finA(6)
            t2_def(6)
            t2_finA(7)
            t2_def(7)

            # unwind pools (stack order per side)
            sp2b[0].release()
            sp2a.release()
            headout_pool.release()
            obuf_pool[0].release()
            wt2_pool.release()
            whead_pool.release()
            xT_pool.release()

    nc.compile()
    return nc


def _get_nc():
    if "nc" not in _CACHE:
        _CACHE["nc"] = _build()
    return _CACHE["nc"]


def _prep_inputs(x, W_head, W_proj1, W_tail1, W_proj2, W_tail2):
    import concourse.mybir as mybir
    f8np = mybir.dt.np(mybir.dt.float8e4)

    def kxn8(w, pad_to=None, scale=WSC):
        # [N, K] weight -> [128, K//128, Np] fp8 (K on partitions), x scale
        n, k = w.shape
        a = np.ascontiguousarray(
            w.T.reshape(k // P, P, n).transpose(1, 0, 2)) * scale
        if pad_to is not None and pad_to != n:
            a = np.concatenate(
                [a, np.zeros((P, k // P, pad_to - n), np.float32)], axis=2)
        return a.astype(f8np)

    x2 = np.asarray(x, np.float32).reshape(T, H)
    xT = np.ascontiguousarray(
        x2.T.reshape(KO_H, P, T).transpose(1, 0, 2)).astype(f8np)
    wcl = kxn8(W_head[20000:20002])
    wp1 = kxn8(W_proj1)
    wp2 = kxn8(W_proj2)

    in_maps = []
    for i in range(N_CORES):
        in_maps.append({
            "xT": xT,
            "wheadT": kxn8(W_head[i * VH:(i + 1) * VH], VHp),
            "wclT": wcl,
            "wp1T": wp1,
            "wp2T": wp2,
            "wt1T": kxn8(W_tail1[i * V1:(i + 1) * V1], V1p),
            "wt2T": kxn8(W_tail2[i * V2:(i + 1) * V2], V2p),
        })
    return in_maps


def _assemble(outs):
    final = np.empty((T, 200000), dtype=np.float32)
    for i in range(N_CORES):
        o = np.asarray(outs[i]["out"])
        final[:, i * VH:(i + 1) * VH] = o[:, :VH]
        final[:, 20000 + i * V1:20000 + (i + 1) * V1] = o[:, VH:VH + V1]
        final[:, 60000 + i * V2:60000 + (i + 1) * V2] = o[:, VH + V1:]
    return final.reshape(2, 512, 200000)


def _run(inputs, trace=False, tmpdir=None):
    from concourse import bass_utils
    nc = _get_nc()
    in_maps = _prep_inputs(**inputs)
    res = bass_utils.run_bass_kernel_spmd(
        nc, in_maps, core_ids=list(range(N_CORES)), trace=trace,
        tmpdir=tmpdir)
    return _assemble(res.results), res


def kernel(**inputs):
    inputs = {k: np.asarray(v) for k, v in inputs.items()}
    out, _ = _run(inputs, trace=False)
    return out


# revision 31
# speedup vs baseline: 1.3198x; 1.0225x over previous
"""Vocab-sharded AdaptiveSoftmax (log_softmax loss head) on 8 TRN2 NeuronCores.

Reference, for x:[2,512,1024] (T=1024 tokens, H=1024):
  head  = x @ W_head.T          -> cols 0:20000 raw logits + 2 cluster logits
  tail1 = cl0 + log_softmax(x @ W_proj1.T @ W_tail1.T)   (40000 vocab)
  tail2 = cl1 + log_softmax(x @ W_proj2.T @ W_tail2.T)   (140000 vocab)
  out   = concat([head[:, :20000], tail1, tail2], -1)

Sharding: vocab dim of head/tail weights split 8 ways (2500/5000/17500 rows
per core, pre-transposed, x32-scaled + fp8e4-cast on host); x replicated.
log_softmax normalizers = AllReduce(add) of per-token exp-sums (the data
distribution keeps |logits| < ~3, so no max-subtraction is needed).

All matmuls run fp8e4 DoubleRow (2 contraction rows per pass); the 1/32
de-scale rides free on ACT `scale` / DVE tensor_scalar. Per 2048-col PSUM
seg: ACT computes exp+accum-sum from PSUM (discard output), and the seg
drains raw logits to a bf16 stage via DVE tensor_scalar_mul (most segs) or
ACT Identity (last 2-3 segs, balancing the engines). Finalize = one DVE
4x-mode tensor_scalar_add of (cl - ln S) over the whole stage. Only
Exp/Ln/Identity are used and the act-table chooser is pinned to the one
set containing all three, so the table loads once.

Phase order P -> T1 -> T2 -> H: tail1's two 4-tile AllReduces resolve
under tail2's first tiles, tail2's per-tile AllReduces resolve 2 tiles
later (finalize issued mid-tile t+2), and the PE-heavy head phase covers
tail2's last finalizes. A dummy warm-up AllReduce during P absorbs the
first-collective latency. Engine split: PE fp8-DR matmuls; DVE drains +
bias adds; ACT exps + a minority of drains; collectives on TOPSP; outputs
and weights on the Sync HWDGE queue, cc staging + g-loads on GpSimd's.
"""

import sys

import numpy as np

if "/opt/trn_rl_repo" not in sys.path:
    sys.path.insert(0, "/opt/trn_rl_repo")

P = 128
T = 1024          # tokens (2*512)
NT = T // P       # 8 token tiles
H = 1024
KO_H = H // P     # 8
VH = 2500         # head vocab shard
VHp = 2512        # padded to %16 for DoubleRow rhs step
V1 = 5000         # tail1 vocab shard
V1p = 5008
V2 = 17500        # tail2 vocab shard
V2p = 17504
E1, E2 = 512, 256
KO_1, KO_2 = E1 // P, E2 // P
C = 512           # matmul free-dim sub-block (psum bank / DR moving limit)
N_CORES = 8
VOUT = VH + V1 + V2   # 25000 per-core out cols
WSC = 32.0        # host-side weight scale into fp8 normal range
ISC = 1.0 / WSC
SEG = 1536        # tail psum tile width (3 f32 banks)

T2SEGS = [(i * SEG, SEG) for i in range(11)] + [(11 * SEG, V2p - 11 * SEG)]
T1SEGS = [(i * SEG, SEG) for i in range(3)] + [(3 * SEG, V1p - 3 * SEG)]
HSUBS = [(0, 512), (512, 512), (1024, 512), (1536, 512), (2048, VHp - 2048)]
NA2 = 8                    # tail2 staged segs per tile; rest deferred
AW2 = NA2 * SEG            # 12288 staged cols
BW2 = V2 - AW2             # 5212 deferred cols (bias fused on recompute)
DEFSEGS = T2SEGS[NA2:]

_CACHE = {}


def _pin_act_tables():
    """Make Exp/Ln resolve only to natural_log_exp_and_others so the ACT
    table loads once instead of thrashing between per-function sets.
    Set ids stay valid: we only shrink the fn sets used by the chooser."""
    import concourse.hw_specs as hw_specs
    import concourse.mybir as mybir
    tabs = hw_specs.get_activation_tables("gen3")  # functools.cached dict
    for name, fns in tabs.items():
        if name != "natural_log_exp_and_others":
            fns.discard(mybir.ActivationFunctionType.Exp)
            fns.discard(mybir.ActivationFunctionType.Ln)


def _build():
    import concourse.bacc as bacc
    import concourse.mybir as mybir
    import concourse.tile as tile
    from contextlib import ExitStack

    _pin_act_tables()

    f8 = mybir.dt.float8e4
    bf16 = mybir.dt.bfloat16
    f32 = mybir.dt.float32
    Exp = mybir.ActivationFunctionType.Exp
    Ident = mybir.ActivationFunctionType.Identity
    Ln = mybir.ActivationFunctionType.Ln
    DR = mybir.MatmulPerfMode.DoubleRow
    AX = mybir.AxisListType.X

    nc = bacc.Bacc("TRN2", target_bir_lowering=False, debug=False,
                   num_devices=N_CORES)

    xT_d = nc.declare_dram_parameter("xT", [P, KO_H, T], f8, False)
    whead_d = nc.declare_dram_parameter("wheadT", [P, KO_H, VHp], f8, False)
    wcl_d = nc.declare_dram_parameter("wclT", [P, KO_H, 2], f8, False)
    wp1_d = nc.declare_dram_parameter("wp1T", [P, KO_H, E1], f8, False)
    wp2_d = nc.declare_dram_parameter("wp2T", [P, KO_H, E2], f8, False)
    wt1_d = nc.declare_dram_parameter("wt1T", [P, KO_1, V1p], f8, False)
    wt2_d = nc.declare_dram_parameter("wt2T", [P, KO_2, V2p], f8, False)
    out_d = nc.declare_dram_parameter("out", [T, VOUT], bf16, True)

    out_r = out_d.ap().rearrange("(t p) v -> p t v", p=P)
    rg = [list(range(N_CORES))]

    with tile.TileContext(nc) as tc:
        with ExitStack() as root:
            pers = root.enter_context(tc.tile_pool(name="pers", bufs=1))
            psum = root.enter_context(
                tc.tile_pool(name="psum", bufs=2, space="PSUM"))
            psumS = root.enter_context(
                tc.tile_pool(name="psumS", bufs=2, space="PSUM"))
            dram = root.enter_context(
                tc.tile_pool(name="dram", bufs=1, space="DRAM"))
            scratch = root.enter_context(tc.tile_pool(name="scratch", bufs=2))

            # persistent small tiles
            p2T = pers.tile([P, KO_2, T], f8, name="p2T")
            cl = pers.tile([P, NT, 2], f32, name="cl")
            s1acc = pers.tile([P, NT, 4], f32, name="s1acc")
            s2acc = pers.tile([P, NT, 12], f32, name="s2acc")
            s1 = pers.tile([P, NT], f32, name="s1")
            s2 = pers.tile([P, NT], f32, name="s2")
            g1 = pers.tile([P, NT], f32, name="g1")
            g2 = pers.tile([P, NT], f32, name="g2")
            b1 = pers.tile([P, NT], f32, name="b1")
            b2 = pers.tile([P, NT], f32, name="b2")
            wrm = pers.tile([P, 1], f32, name="wrm")
            exb = scratch.tile([P, SEG], f8, tag="exb", bufs=1)

            ccw_in = dram.tile([P, 1], f32, name="ccw_in")
            ccw_out = dram.tile([P, 1], f32, name="ccw_out",
                                addr_space="Shared")
            cc1_in = [dram.tile([P, 4], f32, name=f"cc1_in{i}")
                      for i in range(2)]
            cc1_out = [dram.tile([P, 4], f32, name=f"cc1_out{i}",
                                 addr_space="Shared") for i in range(2)]
            cc2_in = [dram.tile([P, 2], f32, name=f"cc2_in{b}")
                      for b in range(NT // 2)]
            cc2_out = [dram.tile([P, 2], f32, name=f"cc2_out{b}",
                                 addr_space="Shared") for b in range(NT // 2)]

            def mm_seg(ps, w, lhsT_sb, kop, t, rhs_sb, voff):
                """Accumulate [128 tokens, w] logits (x32 scale) into psum ps
                for token tile t via DoubleRow fp8: kop k-pairs, rhs columns
                voff:voff+w."""
                for kk in range(kop):
                    for sub in range(0, w, C):
                        sw = min(C, w - sub)
                        nc.tensor.matmul(
                            ps[:, sub:sub + sw],
                            lhsT_sb[:, 2 * kk:2 * kk + 2, t * P:(t + 1) * P],
                            rhs_sb[:, 2 * kk:2 * kk + 2,
                                   voff + sub:voff + sub + sw],
                            start=(kk == 0), stop=(kk == kop - 1),
                            perf_mode=DR)

            # ================= Phase P =================
            xT_pool = tc.alloc_tile_pool(name="xTp", bufs=1, side="right")
            xT = xT_pool.tile([P, KO_H, T], f8, name="xT")
            whead_pool = tc.alloc_tile_pool(name="wheadp", bufs=1,
                                            side="right")
            whead = whead_pool.tile([P, KO_H, VHp], f8, name="whead")
            p1T_pool = tc.alloc_tile_pool(name="p1Tp", bufs=1)
            p1Tl = p1T_pool.tile([P, KO_1, T], f8, name="p1Tl")
            wt1_pool = tc.alloc_tile_pool(name="wt1p", bufs=1)
            wt1 = wt1_pool.tile([P, KO_1, V1p], f8, name="wt1")
            wp_pool = tc.alloc_tile_pool(name="wpp", bufs=1)
            wp1 = wp_pool.tile([P, KO_H, E1], f8, name="wp1")
            wp2 = wp_pool.tile([P, KO_H, E2], f8, name="wp2")
            wcl = wp_pool.tile([P, KO_H, 2], f8, name="wcl")

            nc.sync.dma_start(wp1[:], wp1_d[:])
            nc.sync.dma_start(wcl[:], wcl_d[:])
            nc.sync.dma_start(wp2[:], wp2_d[:])
            nc.sync.dma_start(xT[:], xT_d[:])
            nc.sync.dma_start(wt1[:], wt1_d[:])

            # warm up the collectives pipe (first AR pays ~30us extra)
            nc.vector.memset(wrm[:], 1.0)
            nc.gpsimd.dma_start(ccw_in[:], wrm[:])
            nc.gpsimd.collective_compute(
                "AllReduce", mybir.AluOpType.add, replica_groups=rg,
                ins=[ccw_in[:].opt()], outs=[ccw_out[:].opt()])

            for proj_sb, wp_sb, ko in ((p1Tl, wp1, KO_1), (p2T, wp2, KO_2)):
                for e in range(ko):
                    for th in range(2):
                        ps = psumS.tile([P, 512], f32, tag="hs")
                        for kk in range(KO_H // 2):
                            nc.tensor.matmul(
                                ps[:],
                                wp_sb[:, 2 * kk:2 * kk + 2,
                                      e * P:(e + 1) * P],
                                xT[:, 2 * kk:2 * kk + 2,
                                   th * 512:(th + 1) * 512],
                                start=(kk == 0), stop=(kk == KO_H // 2 - 1),
                                perf_mode=DR)
                        nc.scalar.activation(
                            proj_sb[:, e, th * 512:(th + 1) * 512],
                            ps[:], Ident, scale=ISC)
            for t in range(NT):
                ps = psumS.tile([P, 512], f32, tag="hs")
                for kk in range(KO_H // 2):
                    nc.tensor.matmul(
                        ps[:, :2],
                        xT[:, 2 * kk:2 * kk + 2, t * P:(t + 1) * P],
                        wcl[:, 2 * kk:2 * kk + 2, :],
                        start=(kk == 0), stop=(kk == KO_H // 2 - 1),
                        perf_mode=DR)
                nc.vector.tensor_scalar_mul(cl[:, t, :], ps[:, :2], ISC)
            wp_pool.release()

            wt2_pool = tc.alloc_tile_pool(name="wt2p", bufs=1, side="right")
            wt2 = wt2_pool.tile([P, KO_2, V2p], f8, name="wt2")
            for off, w in T2SEGS:
                nc.sync.dma_start(wt2[:, :, off:off + w],
                                  wt2_d[:, :, off:off + w])
            nc.sync.dma_start(whead[:], whead_d[:])

            # ================= Phase T1 =================
            # tiles 4-7 pool allocated first: it is released later (LIFO)
            sp1L = tc.alloc_tile_pool(name="sp1L", bufs=4, side="right")
            sp1E = tc.alloc_tile_pool(name="sp1E", bufs=4, side="right")
            stg1 = {}

            def t1_compute(t):
                pool = sp1E if t < 4 else sp1L
                stg = pool.tile([P, V1], bf16, name=f"stg1_{t}", tag="s1")
                stg1[t] = stg
                for si, (off, w) in enumerate(T1SEGS):
                    if w > 512:
                        ps = psum.tile([P, SEG], f32, tag="mm")
                    else:
                        ps = psumS.tile([P, 512], f32, tag="hs")
                    mm_seg(ps, w, p1Tl, KO_1 // 2, t, wt1, off)
                    wt = min(w, V1 - off)
                    nc.scalar.activation(
                        exb[:, :wt], ps[:, :wt], Exp, scale=ISC,
                        accum_out=s1acc[:, t, si:si + 1])
                    if si < 3:
                        nc.vector.tensor_scalar_mul(
                            stg[:, off:off + wt], ps[:, :wt], ISC)
                    else:
                        nc.scalar.activation(
                            stg[:, off:off + wt], ps[:, :wt], Ident,
                            scale=ISC)
                nc.vector.reduce_sum(s1[:, t:t + 1], s1acc[:, t, :], axis=AX)

            def t1_ar(i):  # i = batch 0 (tiles 0-3) or 1 (tiles 4-7)
                nc.gpsimd.dma_start(cc1_in[i][:], s1[:, 4 * i:4 * i + 4])
                nc.gpsimd.collective_compute(
                    "AllReduce", mybir.AluOpType.add, replica_groups=rg,
                    ins=[cc1_in[i][:].opt()], outs=[cc1_out[i][:].opt()])

            def t1_bias(i):
                nc.gpsimd.dma_start(g1[:, 4 * i:4 * i + 4], cc1_out[i][:])
                lng = scratch.tile([P, 4], f32, tag="lng4")
                nc.scalar.activation(lng[:], g1[:, 4 * i:4 * i + 4], Ln)
                nc.vector.tensor_sub(out=b1[:, 4 * i:4 * i + 4],
                                     in0=cl[:, 4 * i:4 * i + 4, 0],
                                     in1=lng[:])

            def t1_finalize(t):
                nc.vector.tensor_scalar_add(stg1[t][:], stg1[t][:],
                                            b1[:, t:t + 1])
                nc.sync.dma_start(out_r[:, t, VH:VH + V1], stg1[t][:])

            for t in range(4):
                t1_compute(t)
            t1_ar(0)
            t1_compute(4)
            t1_compute(5)
            t1_compute(6)
            t1_bias(0)
            t1_finalize(0)
            t1_finalize(1)
            t1_compute(7)
            t1_ar(1)
            t1_finalize(2)
            t1_finalize(3)
            sp1E.release()
            wt1_pool.release()
            p1T_pool.release()

            # ================= Phase T2 (head tiles interleaved) ========
            headout_pool = tc.alloc_tile_pool(name="headoutp", bufs=3)
            sp2a = tc.alloc_tile_pool(name="sp2a", bufs=2)
            sp2b = [None]   # allocated after sp1L release
            obuf_pool = [None]
            stg2 = {}

            def t2_compute(t, mids=()):
                mids = dict(mids)
                pool = sp2a if (t // 2) % 2 == 0 else sp2b[0]
                stg = pool.tile([P, AW2], bf16, name=f"stg2_{t}", tag="s2")
                stg2[t] = stg
                for si, (off, w) in enumerate(T2SEGS):
                    for m in mids.get(si, ()):
                        m()
                    ps = psum.tile([P, SEG], f32, tag="mm")
                    mm_seg(ps, w, p2T, KO_2 // 2, t, wt2, off)
                    wt = min(w, V2 - off)
                    nc.scalar.activation(
                        exb[:, :wt], ps[:, :wt], Exp, scale=ISC,
                        accum_out=s2acc[:, t, si:si + 1])
                    if si < NA2:
                        nc.vector.tensor_scalar_mul(
                            stg[:, off:off + wt], ps[:, :wt], ISC)
                nc.vector.reduce_sum(s2[:, t:t + 1], s2acc[:, t, :], axis=AX)
                if t % 2:
                    b = t // 2
                    nc.gpsimd.dma_start(cc2_in[b][:], s2[:, t - 1:t + 1])
                    nc.gpsimd.collective_compute(
                        "AllReduce", mybir.AluOpType.add, replica_groups=rg,
                        ins=[cc2_in[b][:].opt()], outs=[cc2_out[b][:].opt()])

            def t2_bias(b):
                nc.gpsimd.dma_start(g2[:, 2 * b:2 * b + 2], cc2_out[b][:])
                lng = scratch.tile([P, 2], f32, tag="lng2")
                nc.scalar.activation(lng[:], g2[:, 2 * b:2 * b + 2], Ln)
                nc.vector.tensor_sub(out=b2[:, 2 * b:2 * b + 2],
                                     in0=cl[:, 2 * b:2 * b + 2, 1],
                                     in1=lng[:])

            def t2_finA(t):
                # staged region: add bias, ship
                nc.vector.tensor_scalar_add(stg2[t][:], stg2[t][:],
                                            b2[:, t:t + 1])
                nc.sync.dma_start(out_r[:, t, VH + V1:VH + V1 + AW2],
                                  stg2[t][:])

            obufs = {}

            def t2_defH(t, half):
                # deferred region: recompute, drain with fused bias
                if half == 0:
                    obufs[t] = obuf_pool[0].tile([P, BW2], bf16, tag="ob",
                                                 name=f"ob{t}")
                ob = obufs[t]
                for si in (0, 1) if half == 0 else (2, 3):
                    off, w = DEFSEGS[si]
                    ps = psum.tile([P, SEG], f32, tag="mm")
                    mm_seg(ps, w, p2T, KO_2 // 2, t, wt2, off)
                    wt = min(w, V2 - off)
                    o = ob[:, off - AW2:off - AW2 + wt]
                    if half == 0:
                        nc.scalar.activation(o, ps[:, :wt], Ident,
                                             scale=ISC, bias=b2[:, t:t + 1])
                    else:
                        nc.vector.tensor_scalar(
                            o, ps[:, :wt], ISC, b2[:, t:t + 1],
                            mybir.AluOpType.mult, mybir.AluOpType.add)
                if half == 1:
                    nc.sync.dma_start(out_r[:, t, VH + V1 + AW2:VOUT],
                                      ob[:, :BW2])

            # ============ Phase H (instruction-level interleave) ========
            # head subs run on their own 1-bank psum stream, spread into
            # the tail2 seg loop so the PE always has an independent
            # matmul to fill drain-bound gaps.
            hos = {}

            def h_sub(t, k):
                if k == 0:
                    hos[t] = headout_pool.tile([P, VHp], bf16, tag="ho",
                                               name=f"ho{t}")
                ho = hos[t]
                off, w = HSUBS[k]
                ps = psumS.tile([P, 512], f32, tag="hs")
                for kk in range(KO_H // 2):
                    nc.tensor.matmul(
                        ps[:, :w],
                        xT[:, 2 * kk:2 * kk + 2, t * P:(t + 1) * P],
                        whead[:, 2 * kk:2 * kk + 2, off:off + w],
                        start=(kk == 0), stop=(kk == KO_H // 2 - 1),
                        perf_mode=DR)
                if k < 3:
                    nc.vector.tensor_scalar_mul(
                        ho[:, off:off + w], ps[:, :w], ISC)
                else:
                    nc.scalar.activation(ho[:, off:off + w], ps[:, :w],
                                         Ident, scale=ISC)
                if k == 4:
                    nc.sync.dma_start(out_r[:, t, 0:VH], ho[:, :VH])

            def h_tile(t):
                for k in range(5):
                    h_sub(t, k)

            def h_mids(t):
                return {2: (lambda: h_sub(t, 0),),
                        4: (lambda: h_sub(t, 1),),
                        6: (lambda: h_sub(t, 2),),
                        8: (lambda: h_sub(t, 3),),
                        10: (lambda: h_sub(t, 4),)}

            def fin_mids(b):
                # finalize pair {2b, 2b+1} spread through tile 2b+3
                t0, t1x = 2 * b, 2 * b + 1
                return {3: (lambda: t2_bias(b), lambda: t2_finA(t0)),
                        5: (lambda: t2_defH(t0, 0),),
                        7: (lambda: t2_defH(t0, 1), lambda: t2_finA(t1x)),
                        9: (lambda: t2_defH(t1x, 0),),
                        11: (lambda: t2_defH(t1x, 1),)}

            h_tile(0)
            t2_compute(0, mids={5: (lambda: t1_bias(1),
                                    lambda: t1_finalize(4),
                                    lambda: t1_finalize(5),
                                    lambda: t1_finalize(6),
                                    lambda: t1_finalize(7))})
            sp1L.release()
            sp2b[0] = tc.alloc_tile_pool(name="sp2b", bufs=2)
            obuf_pool[0] = tc.alloc_tile_pool(name="obuf", bufs=2,
                                              side="right")
            t2_compute(1, mids=h_mids(1))
            t2_compute(2, mids=h_mids(2))
            t2_compute(3, mids={**h_mids(3), **fin_mids(0)})
            t2_compute(4, mids=h_mids(4))
            t2_compute(5, mids={**h_mids(5), **fin_mids(1)})
            t2_compute(6, mids=h_mids(6))
            t2_compute(7, mids=fin_mids(2))
            h_tile(7)
            t2_bias(3)
            t2_finA(6)
            t2_defH(6, 0)
            t2_defH(6, 1)
            t2_finA(7)
            t2_defH(7, 0)
            t2_defH(7, 1)

            # unwind pools (stack order per side)
            sp2b[0].release()
            sp2a.release()
            headout_pool.release()
            obuf_pool[0].release()
            wt2_pool.release()
            whead_pool.release()
            xT_pool.release()

    nc.compile()
    return nc


def _get_nc():
    if "nc" not in _CACHE:
        _CACHE["nc"] = _build()
    return _CACHE["nc"]


def _prep_inputs(x, W_head, W_proj1, W_tail1, W_proj2, W_tail2):
    import concourse.mybir as mybir
    f8np = mybir.dt.np(mybir.dt.float8e4)

    def kxn8(w, pad_to=None, scale=WSC):
        # [N, K] weight -> [128, K//128, Np] fp8 (K on partitions), x scale
        n, k = w.shape
        a = np.ascontiguousarray(
            w.T.reshape(k // P, P, n).transpose(1, 0, 2)) * scale
        if pad_to is not None and pad_to != n:
            a = np.concatenate(
                [a, np.zeros((P, k // P, pad_to - n), np.float32)], axis=2)
        return a.astype(f8np)

    x2 = np.asarray(x, np.float32).reshape(T, H)
    xT = np.ascontiguousarray(
        x2.T.reshape(KO_H, P, T).transpose(1, 0, 2)).astype(f8np)
    wcl = kxn8(W_head[20000:20002])
    wp1 = kxn8(W_proj1)
    wp2 = kxn8(W_proj2)

    in_maps = []
    for i in range(N_CORES):
        in_maps.append({
            "xT": xT,
            "wheadT": kxn8(W_head[i * VH:(i + 1) * VH], VHp),
            "wclT": wcl,
            "wp1T": wp1,
            "wp2T": wp2,
            "wt1T": kxn8(W_tail1[i * V1:(i + 1) * V1], V1p),
            "wt2T": kxn8(W_tail2[i * V2:(i + 1) * V2], V2p),
        })
    return in_maps


def _assemble(outs):
    final = np.empty((T, 200000), dtype=np.float32)
    for i in range(N_CORES):
        o = np.asarray(outs[i]["out"])
        final[:, i * VH:(i + 1) * VH] = o[:, :VH]
        final[:, 20000 + i * V1:20000 + (i + 1) * V1] = o[:, VH:VH + V1]
        final[:, 60000 + i * V2:60000 + (i + 1) * V2] = o[:, VH + V1:]
    return final.reshape(2, 512, 200000)


def _run(inputs, trace=False, tmpdir=None):
    from concourse import bass_utils
    nc = _get_nc()
    in_maps = _prep_inputs(**inputs)
    res = bass_utils.run_bass_kernel_spmd(
        nc, in_maps, core_ids=list(range(N_CORES)), trace=trace,
        tmpdir=tmpdir)
    return _assemble(res.results), res


def kernel(**inputs):
    inputs = {k: np.asarray(v) for k, v in inputs.items()}
    out, _ = _run(inputs, trace=False)
    return out
